# revision 21
# baseline (speedup 1.0000x reference)
"""Trainium2 Bass kernel for the sparse submanifold 3D CNN (nn_Net_38963943309313).

Network: 7 blocks of 2 submanifold 3x3x3 convs on a 64^3 grid, 2x2x2 sparse
max-pools between blocks, channels 3->64->...->256, output [1,1,1,1,256].

Strategy (8 NeuronCores):
 - Shard z-slabs across cores for levels 0-2 (grids 64/32/16), AllGather the
   pooled activations between levels. Levels 3-6 (grids 8/4/2/1) replicated.
 - Convs are fp32r matmuls: activations channel-major [C, z, y, x] in SBUF
   (y/x zero-padded), 27 shifted-window matmuls accumulated in PSUM.
 - Wire-format optimization (the dominant per-call cost is host->device
   upload through the PJRT tunnel): every core receives just 2 arrays -
     blob16 [DLEN+WSH] fp16: its padded dense-input z-slab [12,3,66,66]
                             followed by 1/8 of the packed conv weights,
     aux8   [AUX8TOT] uint8: the independent 0/1 mask slabs.
   Weights are AllGathered on-device (DRAM->DRAM, fp16) and upcast on load;
   all f32 mask sections (including the (m-1)*3e4 mn* evictors and the m*p
   aliases) are derived on device into a DRAM f32 scratch and broadcast-read
   from there; conv1 of block 0 is an fp16 matmul over an im2col tile built
   on-device by 9 strided DMAs per slice.
 - Instruction-count optimization (per-call executable load costs ~36us per
   static instruction): L0's conv2+pool - the largest instruction mass - runs
   as a tc.For_i hardware loop over z-pairs with register-offset APs. A0
   holds all 10 h1 slices in fp16 (slot s = rows0 h1[s], rows64 h1[s+1]);
   every in-loop matmul keeps tile_position (0,0) (base-64 moving operands
   with register APs miscompute), so the dz=+1 leftover taps read the
   primary rows0 copies of slots z+2/z+3.
 - Submanifold masking: conv1 evictions multiply by a broadcast mask; conv2
   evictions add (mask-1)*3e4 so the following max-pool ignores inactive
   voxels (activations are O(50), so 3e4 dominates); pool result is
   multiplied by the pooled mask.
"""

import sys

sys.path.insert(0, "/opt/trn_rl_repo")

import numpy as np
import concourse.bass as bass
import concourse.tile as tile
from concourse.tile import add_dep_helper
from concourse import bacc, mybir
from concourse.bass_utils import run_bass_kernel_spmd

NC = 8
GRID = 64
BIGH = 3.0e4  # fp16-representable "infinity" for mask eviction
F32 = mybir.dt.float32
F32R = mybir.dt.float32r
F16 = mybir.dt.float16
U8 = mybir.dt.uint8

OFFSETS = [(dz, dy, dx) for dz in (-1, 0, 1) for dy in (-1, 0, 1) for dx in (-1, 0, 1)]
DYDX = [(dy, dx) for dy in (-1, 0, 1) for dx in (-1, 0, 1)]


def _wlayout():
    specs = [
        ("w81", (81, 128)),
        ("w0p", (128, 9, 64)), ("w0l", (128, 9, 64)),
        ("w1p", (128, 9, 96)), ("w1l", (128, 9, 96)),
        ("w1c2_0", (96, 27, 96)),
        ("w2c1_0", (96, 27, 128)), ("w2c2_0", (128, 27, 128)),
        ("w3c1_0", (128, 27, 160)),
        ("w3c2_0", (128, 27, 160)), ("w3c2_1", (32, 27, 160)),
        ("w4c1_0", (128, 27, 192)), ("w4c1_1", (32, 27, 192)),
        ("w4c2_0", (128, 27, 192)), ("w4c2_1", (64, 27, 192)),
        ("w5c1_0", (128, 27, 224)), ("w5c1_1", (64, 27, 224)),
        ("w5c2_0", (128, 27, 224)), ("w5c2_1", (96, 27, 224)),
        ("w6c1_0", (128, 1, 256)), ("w6c1_1", (96, 1, 256)),
        ("w6c2_0", (128, 1, 256)), ("w6c2_1", (128, 1, 256)),
    ]
    off, lay = 0, {}
    for nm, sh in specs:
        lay[nm] = (off, sh)
        off += int(np.prod(sh))
    return lay, off


WLAYOUT, WTOT = _wlayout()
assert WTOT % NC == 0
WSH = WTOT // NC


def _auxlayout():
    specs = [
        ("m0mul", 10 * 4096), ("mn0", 8 * 4096),
        ("m1p", 4 * 1024), ("m1mul", 6 * 1024), ("mn1", 4 * 1024),
        ("m2p", 2 * 256), ("m2mul", 4 * 256), ("mn2", 2 * 256),
        ("m3p", 64), ("m3mul", 10 * 64), ("mn3", 8 * 64),
        ("m4p", 64), ("m4mul", 64), ("mn4", 64),
        ("m5p", 8), ("m5mul", 8), ("mn5", 8),
        ("m6p", 1),
    ]
    off, lay = 0, {}
    for nm, n in specs:
        lay[nm] = off
        off += n
    tot = ((off + 127) // 128) * 128
    return lay, tot


AUXL, AUXTOT = _auxlayout()
AUXC = AUXTOT // 128


def _aux8layout():
    # wire format: only the independent 0/1 masks; everything else (the
    # m*p aliases and the (m-1)*BIG mn* tensors) is derived on device.
    specs = [("m0mul", 10 * 4096), ("m1mul", 6 * 1024), ("m2mul", 4 * 256),
             ("m3mul", 10 * 64), ("m3p", 64), ("m45", 64), ("m5", 8), ("m6", 1)]
    off, lay = 0, {}
    for nm, n in specs:
        lay[nm] = off
        off += n
    return lay, ((off + 127) // 128) * 128


AUX8L, AUX8TOT = _aux8layout()

DOFF, DLEN = 0, 12 * 3 * 66 * 66     # dense slab inside blob16
WOFF = DLEN                          # weight shard inside blob16
BLOBTOT = DLEN + WSH


def build_host_inputs(features, coors, Ws):
    """Host-side data marshalling. Returns (in_maps, meta)."""
    z, y, x = coors[:, 0], coors[:, 1], coors[:, 2]
    dense = np.zeros((GRID, GRID, GRID, 3), np.float32)
    mask0 = np.zeros((GRID, GRID, GRID), np.float32)
    dense[z, y, x] = features  # last write wins (matches XLA CPU scatter)
    mask0[z, y, x] = 1.0

    # mask pyramid
    masks = [mask0]
    m = mask0
    for _ in range(6):
        mr = m.reshape(m.shape[0] // 2, 2, m.shape[1] // 2, 2, m.shape[2] // 2, 2)
        m = mr.max(axis=(1, 3, 5))
        masks.append(m)

    # padded dense fp16 [68, 3, 66, 66]; core k ships slab [8k, 8k+12)
    # (padded-z index = global z + 2)
    dp = np.zeros((GRID + 4, 3, GRID + 2, GRID + 2), np.float16)
    dp[2:-2, :, 1:-1, 1:-1] = dense.transpose(0, 3, 1, 2).astype(np.float16)
    dense16 = [np.ascontiguousarray(dp[8 * k:8 * k + 12]) for k in range(NC)]

    # ---- weight pack (fp16, single copy, sharded 1/8 per core) ----
    W0 = Ws[0]  # [3,3,3,3,64]
    # im2col row order (ky kx kz ci) to match the on-device xs build
    w81 = W0.transpose(1, 2, 0, 3, 4).reshape(81, 64)
    w81 = np.concatenate([w81, w81], axis=1)  # [81, 128] co-dup

    def pack_pair(W):  # [3,3,3,cin,co] -> pair [2*cin, 9, co] + left [cin, 9, co]
        cin, co = W.shape[3], W.shape[4]
        wp = np.zeros((2 * cin, 9, co), np.float32)
        wl = np.zeros((cin, 9, co), np.float32)
        for j, (dy, dx) in enumerate(DYDX):
            wp[0:cin, j] = W[0, dy + 1, dx + 1]
            wp[cin:2 * cin, j] = W[1, dy + 1, dx + 1]
            wl[:, j] = W[2, dy + 1, dx + 1]
        return wp, wl

    w0p, w0l = pack_pair(Ws[1])
    w1p, w1l = pack_pair(Ws[2])
    w0l = np.concatenate([w0l, w0l], axis=0)  # [128, 9, 64]
    w1l = np.concatenate([w1l, w1l], axis=0)  # [128, 9, 96]

    def pack_generic(W):  # -> list of [kchunk, 27, co]
        cin, co = W.shape[3], W.shape[4]
        wf = W.reshape(27, cin, co)
        return [np.ascontiguousarray(wf[:, k0:k0 + min(128, cin - k0), :]
                                     .transpose(1, 0, 2))
                for k0 in range(0, cin, 128)]

    wtensors = {"w81": w81, "w0p": w0p, "w0l": w0l, "w1p": w1p, "w1l": w1l}
    for li, wi in [("w1c2", 3), ("w2c1", 4), ("w2c2", 5), ("w3c1", 6),
                   ("w3c2", 7), ("w4c1", 8), ("w4c2", 9), ("w5c1", 10),
                   ("w5c2", 11)]:
        for ci, arr in enumerate(pack_generic(Ws[wi])):
            wtensors[f"{li}_{ci}"] = arr
    for li, wi in [("w6c1", 12), ("w6c2", 13)]:
        W = Ws[wi]
        cin = W.shape[3]
        wc = W[1, 1, 1]  # [cin, co]
        for ci, k0 in enumerate(range(0, cin, 128)):
            wtensors[f"{li}_{ci}"] = np.ascontiguousarray(
                wc[k0:k0 + min(128, cin - k0)][:, None, :])

    W16 = np.zeros(WTOT, np.float16)
    for nm, (off, sh) in WLAYOUT.items():
        arr = wtensors[nm]
        assert tuple(arr.shape) == tuple(sh), (nm, arr.shape, sh)
        n = int(np.prod(sh))
        W16[off:off + n] = arr.astype(np.float16).reshape(-1)
    wshards = [np.ascontiguousarray(W16[k * WSH:(k + 1) * WSH])
               for k in range(NC)]

    # ---- aux mask pack (uint8 0/1 wire format, per core) ----
    def slab_mask(mask, z0, nsl):
        D2 = mask.shape[1] * mask.shape[2]
        out = np.zeros((nsl, D2), np.float32)
        for i in range(nsl):
            zg = z0 + i
            if 0 <= zg < mask.shape[0]:
                out[i] = mask[zg].reshape(-1)
        return out

    def mkaux8(k):
        parts = [
            slab_mask(masks[0], 8 * k - 1, 10),   # m0mul
            slab_mask(masks[1], 4 * k - 1, 6),    # m1mul
            slab_mask(masks[2], 2 * k - 1, 4),    # m2mul
            slab_mask(masks[3], -1, 10),          # m3mul
            slab_mask(masks[3], k, 1),            # m3p
            masks[4].reshape(1, -1),              # m45
            masks[5].reshape(1, -1),              # m5
            masks[6].reshape(1, -1),              # m6
        ]
        flat = np.concatenate([p.reshape(-1) for p in parts])
        out = np.zeros(AUX8TOT, np.uint8)
        out[:flat.size] = flat.astype(np.uint8)
        return out

    meta = {
        "mask_flags": {
            lvl: not np.all(masks[lvl] == 1.0) for lvl in range(1, 7)
        },
    }

    in_maps = [{"blob16": np.concatenate([dense16[k].reshape(-1), wshards[k]]),
                "aux8": mkaux8(k)} for k in range(NC)]
    return in_maps, meta


def build_kernel(meta):
    import contextlib
    nc = bacc.Bacc("TRN2", target_bir_lowering=False, debug=False, num_devices=NC)
    mf = meta["mask_flags"]

    blob16_d = nc.dram_tensor("blob16", [BLOBTOT], F16, kind="ExternalInput")
    aux8_d = nc.dram_tensor("aux8", [AUX8TOT], U8, kind="ExternalInput")
    out_d = nc.dram_tensor("out", [1, 256], F32, kind="ExternalOutput")

    import os as _os
    DBG = bool(_os.environ.get("K_DEBUG"))
    dbg_d = {}
    if DBG:
        for nm, sh in [("dP0", (64, 4, 1156)), ("dA1", (128, 8, 1156)),
                       ("dB1", (96, 6, 1156)), ("dC1", (96, 4, 1024)),
                       ("dP1", (96, 2, 324)), ("dA2", (96, 6, 324)),
                       ("dB2", (128, 4, 324)), ("dC2", (128, 2, 256)),
                       ("dP2", (128, 1, 100)), ("dA3", (128, 12, 100)),
                       ("dB3a", (128, 10, 100)), ("dC3a", (128, 512)),
                       ("dP4a", (128, 216)), ("dB4a", (128, 216)),
                       ("dC4a", (128, 64)), ("dP5a", (128, 64)),
                       ("dB5a", (128, 64)), ("dP6a", (128, 27)),
                       ("dX6a", (128, 1)), ("dC0", (64, 2, 4096)),
                       ("dXS", (81, 4096))]:
            dbg_d[nm] = nc.dram_tensor(nm, list(sh), F32, kind="ExternalOutput")

    with tile.TileContext(nc) as tc:
        ctx = contextlib.ExitStack()
        with ctx:
            pst = ctx.enter_context(tc.tile_pool(name="ps", bufs=4, space="PSUM"))
            drm = ctx.enter_context(tc.tile_pool(name="dram", bufs=1, space="DRAM"))
            glob = ctx.enter_context(tc.tile_pool(name="glob", bufs=1))

            pid = nc.sync.partition_id()

            # ---------- weight shard AllGather + aux mask upcast ----------
            WST = drm.tile([WSH], F16)
            GW = drm.tile([WTOT], F16)
            AUXF = drm.tile([AUXTOT], F32)

            cpw = nc.sync.dma_start(WST[:], blob16_d[WOFF:WOFF + WSH])
            agw = nc.gpsimd.collective_compute(
                "AllGather", mybir.AluOpType.bypass,
                replica_groups=[list(range(NC))],
                ins=[WST[:].opt()], outs=[GW[:].opt()])
            add_dep_helper(agw.ins, cpw.ins, reason="stage shard before gather")

            # dense slab view inside the blob
            DNS = blob16_d[DOFF:DOFF + DLEN].rearrange(
                "(z c a b) -> z c a b", c=3, a=66, b=66)

            # derive all f32 mask sections (and mn* = (m-1)*BIGH) from the
            # uint8 wire masks into the AUXF DRAM scratch
            AUXWB = {}
            with tc.tile_pool(name="prep", bufs=2) as prep:
                def aux_sec(dst, src_off, n, affine):
                    p = 128 if n % 128 == 0 else n
                    c = n // p
                    tu = prep.tile([p, c], U8, tag="a8")
                    nc.sync.dma_start(
                        tu[:], aux8_d[src_off:src_off + n]
                        .rearrange("(p c) -> p c", c=c))
                    tf = prep.tile([p, c], F32, tag="af")
                    if affine:
                        tc_ = prep.tile([p, c], F32, tag="ac")
                        nc.vector.tensor_copy(tc_[:], tu[:])
                        nc.vector.tensor_scalar(tf[:], tc_[:], BIGH, -BIGH,
                                                mybir.AluOpType.mult,
                                                mybir.AluOpType.add)
                    else:
                        nc.vector.tensor_copy(tf[:], tu[:])
                    AUXWB[dst] = nc.sync.dma_start(
                        AUXF[AUXL[dst]:AUXL[dst] + n]
                        .rearrange("(p c) -> p c", c=c), tf[:])

                aux_sec("m0mul", AUX8L["m0mul"], 40960, False)
                aux_sec("mn0", AUX8L["m0mul"] + 4096, 32768, True)
                aux_sec("m1p", AUX8L["m1mul"] + 1024, 4096, False)
                aux_sec("m1mul", AUX8L["m1mul"], 6144, False)
                aux_sec("mn1", AUX8L["m1mul"] + 1024, 4096, True)
                aux_sec("m2p", AUX8L["m2mul"] + 256, 512, False)
                aux_sec("m2mul", AUX8L["m2mul"], 1024, False)
                aux_sec("mn2", AUX8L["m2mul"] + 256, 512, True)
                aux_sec("m3p", AUX8L["m3p"], 64, False)
                aux_sec("m3mul", AUX8L["m3mul"], 640, False)
                aux_sec("mn3", AUX8L["m3mul"] + 64, 512, True)
                aux_sec("m4p", AUX8L["m45"], 64, False)
                aux_sec("m4mul", AUX8L["m45"], 64, False)
                aux_sec("mn4", AUX8L["m45"], 64, True)
                aux_sec("m5p", AUX8L["m5"], 8, False)
                aux_sec("m5mul", AUX8L["m5"], 8, False)
                aux_sec("mn5", AUX8L["m5"], 8, True)
                aux_sec("m6p", AUX8L["m6"], 1, False)

            def mload(sm, sec, rel, n, parts, tag):
                mt = sm.tile([parts, n], F32, tag=tag)
                off = AUXL[sec] + rel
                r = nc.sync.dma_start(
                    mt[:], AUXF[off:off + n].unsqueeze(0).to_broadcast((parts, n)))
                add_dep_helper(r.ins, AUXWB[sec].ins, reason="aux masks written")
                return mt

            def wload(pool, spool, nm, dt=F32R, name=None):
                off, sh = WLAYOUT[nm]
                p, c = sh[0], int(np.prod(sh[1:]))
                t16 = spool.tile([p, c], F16, tag="wst", name=f"wst_{nm}")
                r = nc.sync.dma_start(
                    t16[:], GW[off:off + p * c].rearrange("(p c) -> p c", c=c))
                add_dep_helper(r.ins, agw.ins, reason="weights gathered")
                t = pool.tile(list(sh), dt, name=name or f"sb_{nm}")
                dv = t[:]
                if len(sh) == 3:
                    dv = dv.rearrange("p a b -> p (a b)")
                nc.vector.tensor_copy(dv, t16[:])
                return t

            def wload16raw(pool, nm):
                off, sh = WLAYOUT[nm]
                p, c = sh[0], int(np.prod(sh[1:]))
                t = pool.tile(list(sh), F16, name=f"sb_{nm}")
                dv = t[:]
                if len(sh) == 3:
                    dv = dv.rearrange("p a b -> p (a b)")
                r = nc.sync.dma_start(
                    dv, GW[off:off + p * c].rearrange("(p c) -> p c", c=c))
                add_dep_helper(r.ins, agw.ins, reason="weights gathered")
                return t

            # zero tile for G-pad zeroing
            zt = glob.tile([128, 1156], F32)
            nc.vector.memset(zt[:], 0.0)

            # DRAM gather buffers
            c1_d = drm.tile([4, 64, 1156], F32)
            G1 = drm.tile([36, 64, 1156], F32)
            c2_d = drm.tile([2, 96, 324], F32)
            G2 = drm.tile([20, 96, 324], F32)
            c3_d = drm.tile([1, 128, 100], F32)
            G3 = drm.tile([12, 128, 100], F32)
            gpad_insts = []
            for G, csz, npad in ((G1, (64, 1156), 2), (G2, (96, 324), 2),
                                 (G3, (128, 100), 2)):
                n = G.shape[0]
                for s in list(range(npad)) + list(range(n - npad, n)):
                    gpad_insts.append(
                        nc.sync.dma_start(G[s], zt[0:csz[0], 0:csz[1]]))

            # persistent tail tensors
            P4a = glob.tile([128, 216], F32); P4b = glob.tile([32, 216], F32)
            P5a = glob.tile([128, 64], F32); P5b = glob.tile([64, 64], F32)
            P6a = glob.tile([128, 27], F32); P6b = glob.tile([96, 27], F32)
            X6a = glob.tile([128, 1], F32); X6b = glob.tile([128, 1], F32)
            outt = glob.tile([128, 2], F32)
            for t in (P4a, P4b, P5a, P5b, P6a, P6b):
                nc.vector.memset(t[:].bitcast(F32), 0.0)

            # ================ LEVEL 0 ================
            with tc.tile_pool(name="l0w", bufs=1) as wp, \
                 tc.tile_pool(name="l0st", bufs=2) as sp, \
                 tc.tile_pool(name="l0p", bufs=1) as pp, \
                 tc.tile_pool(name="l0s", bufs=2) as ss, \
                 tc.tile_pool(name="l0m", bufs=4) as sm:
                w81_t = wload16raw(wp, "w81")
                w0p_t = wload16raw(wp, "w0p")   # [128, 9, 64] f16
                w0l_t = wload16raw(wp, "w0l")

                # A0: 10 h1 slices, fp16, slot s = (rows0: h1[s], rows64: h1[s+1])
                A0 = pp.tile([128, 10, 4356], F16)
                C0 = pp.tile([64, 2, 4096], F32R)
                P0 = pp.tile([64, 4, 1156], F32R)
                nc.vector.memset(A0[:], 0.0)
                nc.vector.memset(P0[:].bitcast(F32), 0.0)

                def l0_conv1(sl):
                    # on-device im2col: xs rows = 9*(ky*3+kx) + (kz*3+ci)
                    xs = ss.tile([81, 4096], F16, tag="x1s")
                    for jj, (dy, dx) in enumerate(DYDX):
                        nc.sync.dma_start(
                            xs[9 * jj:9 * jj + 9].rearrange(
                                "p (a b) -> p a b", b=64),
                            DNS[sl:sl + 3, :, 1 + dy:65 + dy,
                                1 + dx:65 + dx]
                            .rearrange("z c a b -> (z c) a b"))
                    for chunk in range(8):
                        ps = pst.tile([128, 512], F32, tag="ps")
                        nc.tensor.matmul(ps[:], w81_t[:],
                                         xs[:, chunk * 512:chunk * 512 + 512],
                                         start=True, stop=True)
                        mt = mload(sm, "m0mul", sl * 4096 + chunk * 512,
                                   512, 128, "m0")
                        yb = chunk * 8
                        d0 = A0[0:64, sl, :].rearrange("p (a b) -> p a b", b=66)
                        nc.vector.tensor_mul(
                            d0[:, yb + 1:yb + 9, 1:65],
                            ps[0:64, :].rearrange("p (a b) -> p a b", b=64),
                            mt[0:64, :].rearrange("p (a b) -> p a b", b=64))
                        if sl >= 1:
                            d1 = A0[64:128, sl - 1, :].rearrange(
                                "p (a b) -> p a b", b=66)
                            nc.vector.tensor_mul(
                                d1[:, yb + 1:yb + 9, 1:65],
                                ps[64:128, :].rearrange("p (a b) -> p a b", b=64),
                                mt[64:128, :].rearrange("p (a b) -> p a b", b=64))

                for sl in range(10):
                    l0_conv1(sl)

                # conv2 + pool: hardware loop over z-pairs (zv = 0,2,4,6).
                # A0r rows = slot*66 + y; out zv pair=slot zv, out zv+1
                # pair=slot zv+1; dz=+1 leftovers h1[zv+2]/h1[zv+3] read from
                # the primary copies (rows 0:64 of slots zv+2 / zv+3) so every
                # register-offset matmul stays at tile_position (0, 0) —
                # base-64 moving operands with register APs produce NaNs.
                A0r = A0[:].rearrange("p s (a b) -> p (s a) b", b=66)
                A0l = A0[0:64, :, :].rearrange("p s (a b) -> p (s a) b", b=66)
                P0r = P0[:].rearrange("p s (a b) -> p (s a) b", b=34)
                with tc.For_i(0, 8, 2) as zv:
                    for chunk in range(8):
                        yb = chunk * 8
                        psA = pst.tile([64, 512], F32, tag="ps")
                        psB = pst.tile([64, 512], F32, tag="ps")
                        for j, (dy, dx) in enumerate(DYDX):
                            first, last = (j == 0), (j == 8)
                            xsl = slice(1 + dx, 65 + dx)
                            vA = psA[:].rearrange("p (a b) -> p a b", b=64)
                            vB = psB[:].rearrange("p (a b) -> p a b", b=64)
                            rbase = zv * 66 + (yb + 1 + dy)
                            nc.tensor.matmul(
                                vA, w0p_t[:, j, :],
                                A0r[:, bass.ds(rbase, 8), xsl],
                                start=first, stop=False,
                                tile_position=(0, 0))
                            nc.tensor.matmul(
                                vB, w0p_t[:, j, :],
                                A0r[:, bass.ds(rbase + 66, 8), xsl],
                                start=first, stop=False,
                                tile_position=(0, 0))
                            nc.tensor.matmul(
                                vA, w0l_t[0:64, j, :],
                                A0l[:, bass.ds(rbase + 132, 8), xsl],
                                start=False, stop=last,
                                tile_position=(0, 0))
                            nc.tensor.matmul(
                                vB, w0l_t[0:64, j, :],
                                A0l[:, bass.ds(rbase + 198, 8), xsl],
                                start=False, stop=last,
                                tile_position=(0, 0))
                        for ps_, h in ((psA, 0), (psB, 1)):
                            mt = sm.tile([64, 512], F32, tag="mn0")
                            r = nc.sync.dma_start(
                                mt[:],
                                AUXF[bass.ds(
                                    AUXL["mn0"] + zv * 4096 + h * 4096 + yb * 64,
                                    512)]
                                .unsqueeze(0).to_broadcast((64, 512)))
                            add_dep_helper(r.ins, AUXWB["mn0"].ins,
                                           reason="aux masks written")
                            nc.vector.tensor_add(
                                C0[:, h, yb * 64:yb * 64 + 512], ps_[:], mt[:])
                    # pool z-pair -> P0 slot zv/2
                    nc.vector.tensor_max(C0[:, 0, :], C0[:, 0, :], C0[:, 1, :])
                    v = C0[:, 0, :].rearrange("p (a b) -> p a b", b=64)
                    t2 = ss.tile([64, 32, 64], F32R, tag="pool0b", bufs=1)
                    nc.vector.tensor_max(t2[:], v[:, 0::2, :], v[:, 1::2, :])
                    t3 = ss.tile([64, 32, 32], F32R, tag="pool0c", bufs=1)
                    nc.vector.tensor_max(t3[:], t2[:, :, 0::2], t2[:, :, 1::2])
                    mtp = sm.tile([64, 1024], F32, tag="m1p")
                    rp = nc.sync.dma_start(
                        mtp[:],
                        AUXF[bass.ds(AUXL["m1p"] + zv * 512, 1024)]
                        .unsqueeze(0).to_broadcast((64, 1024)))
                    add_dep_helper(rp.ins, AUXWB["m1p"].ins,
                                   reason="aux masks written")
                    nc.vector.tensor_mul(
                        P0r[:, bass.ds(zv * 17 + 1, 32), 1:33],
                        t3[:],
                        mtp[:].rearrange("p (a b) -> p a b", b=32))

                if DBG:
                    nc.sync.dma_start(dbg_d["dP0"][:], P0[:].bitcast(F32))
                    nc.sync.dma_start(dbg_d["dC0"][:], C0[:].bitcast(F32))
                nc.sync.dma_start(c1_d[:].rearrange("z c v -> c z v"),
                                  P0[:].bitcast(F32))

            # ---- AllGather L0 -> L1 ----
            ag1 = nc.gpsimd.collective_compute(
                "AllGather", mybir.AluOpType.bypass,
                replica_groups=[list(range(NC))],
                ins=[c1_d[:].opt()], outs=[G1[2:34].opt()])
            for gi in gpad_insts:
                add_dep_helper(ag1.ins, gi.ins, reason="G pads zeroed before gathers")

            # ================ LEVEL 1 ================
            with tc.tile_pool(name="l1w", bufs=1) as wp, \
                 tc.tile_pool(name="l1st", bufs=2) as sp, \
                 tc.tile_pool(name="l1p", bufs=1) as pp, \
                 tc.tile_pool(name="l1s", bufs=2) as ss, \
                 tc.tile_pool(name="l1m", bufs=4) as sm:
                w1p_t = wload(wp, sp, "w1p")
                w1l_t = wload(wp, sp, "w1l")
                w1c2_t = wload(wp, sp, "w1c2_0")

                A1 = pp.tile([128, 8, 1156], F32R)
                B1 = pp.tile([96, 6, 1156], F32R)
                C1 = pp.tile([96, 4, 1024], F32R)
                P1 = pp.tile([96, 2, 324], F32R)
                nc.vector.memset(B1[:].bitcast(F32), 0.0)
                nc.vector.memset(P1[:].bitcast(F32), 0.0)
                _r1 = nc.sync.dma_start(
                    A1[0:64, :, :],
                    G1[bass.ds(pid * 4, 8)].rearrange("z c v -> c z v").bitcast(F32R))
                _r2 = nc.sync.dma_start(
                    A1[64:128, 0:7, :],
                    G1[bass.ds(pid * 4 + 1, 7)].rearrange("z c v -> c z v").bitcast(F32R))
                add_dep_helper(_r1.ins, ag1.ins, reason="gather before dynamic read")
                add_dep_helper(_r2.ins, ag1.ins, reason="gather before dynamic read")

                def l1_conv1(sl):
                    mt = mload(sm, "m1mul", sl * 1024, 1024, 96, "m1mul")
                    pss = [pst.tile([96, 512], F32, tag="ps",
                                    name=f"ps_l1_{sl}_{_c}") for _c in range(2)]
                    wA = A1[:, sl, :].rearrange("p (a b) -> p a b", b=34)
                    wB = A1[64:128, sl + 1, :].rearrange("p (a b) -> p a b", b=34)
                    wC = A1[0:64, sl + 2, :].rearrange("p (a b) -> p a b", b=34)
                    for j, (dy, dx) in enumerate(DYDX):
                        xsl = slice(1 + dx, 33 + dx)
                        for chunk in range(2):
                            yb = chunk * 16
                            ys = slice(yb + 1 + dy, yb + 17 + dy)
                            nc.tensor.matmul(
                                pss[chunk][:].rearrange("p (a b) -> p a b", b=32),
                                w1p_t[:, j, :], wA[:, ys, xsl],
                                start=(j == 0), stop=False)
                        ys0 = slice(1 + dy, 17 + dy)
                        ys1 = slice(17 + dy, 33 + dy)
                        nc.tensor.matmul(
                            pss[0][:].rearrange("p (a b) -> p a b", b=32),
                            w1l_t[64:128, j, :], wB[:, ys0, xsl],
                            start=False, stop=(j == 8))
                        nc.tensor.matmul(
                            pss[1][:].rearrange("p (a b) -> p a b", b=32),
                            w1l_t[0:64, j, :], wC[:, ys1, xsl],
                            start=False, stop=(j == 8))
                    for chunk in range(2):
                        yb = chunk * 16
                        dst = B1[:, sl, :].rearrange("p (a b) -> p a b", b=34)
                        nc.vector.tensor_mul(
                            dst[:, yb + 1:yb + 17, 1:33],
                            pss[chunk][:].rearrange("p (a b) -> p a b", b=32),
                            mt[:, yb * 32:yb * 32 + 512].rearrange(
                                "p (a b) -> p a b", b=32))

                def l1_conv2(sl):
                    mt = mload(sm, "mn1", sl * 1024, 1024, 96, "mn1")
                    for chunk in range(2):
                        yb = chunk * 16
                        ps = pst.tile([96, 512], F32, tag="ps")
                        for o, (dz, dy, dx) in enumerate(OFFSETS):
                            w = B1[:, sl + 1 + dz, :].rearrange(
                                "p (a b) -> p a b", b=34)
                            nc.tensor.matmul(
                                ps[:].rearrange("p (a b) -> p a b", b=32),
                                w1c2_t[:, o, :],
                                w[:, yb + 1 + dy:yb + 17 + dy, 1 + dx:33 + dx],
                                start=(o == 0), stop=(o == 26))
                        nc.vector.tensor_add(C1[:, sl, yb * 32:yb * 32 + 512],
                                             ps[:],
                                             mt[:, yb * 32:yb * 32 + 512])

                def l1_pool(zz):
                    zp = zz // 2
                    nc.vector.tensor_max(C1[:, zz, :], C1[:, zz, :], C1[:, zz + 1, :])
                    v = C1[:, zz, :].rearrange("p (a b) -> p a b", b=32)
                    t2 = ss.tile([96, 16, 32], F32R, tag="pool1b")
                    nc.vector.tensor_max(t2[:], v[:, 0::2, :], v[:, 1::2, :])
                    t3 = ss.tile([96, 16, 16], F32R, tag="pool1c")
                    nc.vector.tensor_max(t3[:], t2[:, :, 0::2], t2[:, :, 1::2])
                    mt = mload(sm, "m2p", zp * 256, 256, 96, "m2p")
                    dst = P1[:, zp, :].rearrange("p (a b) -> p a b", b=18)
                    nc.vector.tensor_mul(
                        dst[:, 1:17, 1:17], t3[:],
                        mt[:].rearrange("p (a b) -> p a b", b=16))

                for sl in range(6):
                    l1_conv1(sl)
                    if sl >= 2:
                        l1_conv2(sl - 2)
                        if sl >= 3 and (sl - 3) % 2 == 0:
                            l1_pool(sl - 3)

                if DBG:
                    nc.sync.dma_start(dbg_d["dA1"][:], A1[:].bitcast(F32))
                    nc.sync.dma_start(dbg_d["dB1"][:], B1[:].bitcast(F32))
                    nc.sync.dma_start(dbg_d["dC1"][:], C1[:].bitcast(F32))
                    nc.sync.dma_start(dbg_d["dP1"][:], P1[:].bitcast(F32))
                nc.sync.dma_start(c2_d[:].rearrange("z c v -> c z v"),
                                  P1[:].bitcast(F32))

            # ---- AllGather L1 -> L2 ----
            ag2 = nc.gpsimd.collective_compute(
                "AllGather", mybir.AluOpType.bypass,
                replica_groups=[list(range(NC))],
                ins=[c2_d[:].opt()], outs=[G2[2:18].opt()])
            for gi in gpad_insts:
                add_dep_helper(ag2.ins, gi.ins, reason="G pads zeroed before gathers")

            # ================ LEVEL 2 ================
            with tc.tile_pool(name="l2w", bufs=1) as wp, \
                 tc.tile_pool(name="l2st", bufs=2) as sp, \
                 tc.tile_pool(name="l2p", bufs=1) as pp, \
                 tc.tile_pool(name="l2s", bufs=2) as ss, \
                 tc.tile_pool(name="l2m", bufs=4) as sm:
                w2c1_t = wload(wp, sp, "w2c1_0")
                w2c2_t = wload(wp, sp, "w2c2_0")
                A2 = pp.tile([96, 6, 324], F32R)
                B2 = pp.tile([128, 4, 324], F32R)
                C2 = pp.tile([128, 2, 256], F32R)
                P2 = pp.tile([128, 1, 100], F32R)
                nc.vector.memset(B2[:].bitcast(F32), 0.0)
                nc.vector.memset(P2[:].bitcast(F32), 0.0)
                _r3 = nc.sync.dma_start(
                    A2[:], G2[bass.ds(pid * 2, 6)].rearrange("z c v -> c z v").bitcast(F32R))
                add_dep_helper(_r3.ins, ag2.ins, reason="gather before dynamic read")

                for s0 in (0, 2):
                    ps = pst.tile([128, 512], F32, tag="ps")
                    for o, (dz, dy, dx) in enumerate(OFFSETS):
                        w = A2[:].rearrange("p z (a b) -> p z a b", b=18)
                        nc.tensor.matmul(
                            ps[:].rearrange("p (z a b) -> p z a b", z=2, a=16),
                            w2c1_t[:, o, :],
                            w[:, s0 + dz + 1:s0 + dz + 3,
                              1 + dy:17 + dy, 1 + dx:17 + dx],
                            start=(o == 0), stop=(o == 26))
                    mt = mload(sm, "m2mul", s0 * 256, 512, 128, "m2mul")
                    dst = B2[:].rearrange("p z (a b) -> p z a b", b=18)
                    nc.vector.tensor_mul(
                        dst[:, s0:s0 + 2, 1:17, 1:17],
                        ps[:].rearrange("p (z a b) -> p z a b", z=2, a=16),
                        mt[:].rearrange("p (z a b) -> p z a b", z=2, a=16))

                ps = pst.tile([128, 512], F32, tag="ps")
                for o, (dz, dy, dx) in enumerate(OFFSETS):
                    w = B2[:].rearrange("p z (a b) -> p z a b", b=18)
                    nc.tensor.matmul(
                        ps[:].rearrange("p (z a b) -> p z a b", z=2, a=16),
                        w2c2_t[:, o, :],
                        w[:, dz + 1:dz + 3, 1 + dy:17 + dy, 1 + dx:17 + dx],
                        start=(o == 0), stop=(o == 26))
                if mf[2]:
                    mt = mload(sm, "mn2", 0, 512, 128, "mn2")
                    nc.vector.tensor_add(C2[:].rearrange("p a b -> p (a b)"),
                                         ps[:], mt[:])
                else:
                    nc.scalar.copy(C2[:].rearrange("p a b -> p (a b)"), ps[:])

                # L2 pool
                nc.vector.tensor_max(C2[:, 0, :], C2[:, 0, :], C2[:, 1, :])
                v = C2[:, 0, :].rearrange("p (a b) -> p a b", b=16)
                t2 = ss.tile([128, 8, 16], F32R, tag="pool2b")
                nc.vector.tensor_max(t2[:], v[:, 0::2, :], v[:, 1::2, :])
                dst = P2[:, 0, :].rearrange("p (a b) -> p a b", b=10)
                if mf[3]:
                    t3 = ss.tile([128, 8, 8], F32R, tag="pool2c")
                    nc.vector.tensor_max(t3[:], t2[:, :, 0::2], t2[:, :, 1::2])
                    mt = mload(sm, "m3p", 0, 64, 128, "m3p")
                    nc.vector.tensor_mul(
                        dst[:, 1:9, 1:9], t3[:],
                        mt[:].rearrange("p (a b) -> p a b", b=8))
                else:
                    nc.vector.tensor_max(dst[:, 1:9, 1:9],
                                         t2[:, :, 0::2], t2[:, :, 1::2])

                if DBG:
                    nc.sync.dma_start(dbg_d["dA2"][:], A2[:].bitcast(F32))
                    nc.sync.dma_start(dbg_d["dB2"][:], B2[:].bitcast(F32))
                    nc.sync.dma_start(dbg_d["dC2"][:], C2[:].bitcast(F32))
                    nc.sync.dma_start(dbg_d["dP2"][:], P2[:].bitcast(F32))
                nc.sync.dma_start(c3_d[:].rearrange("z c v -> c z v"),
                                  P2[:].bitcast(F32))

            # ---- AllGather L2 -> L3 ----
            ag3 = nc.gpsimd.collective_compute(
                "AllGather", mybir.AluOpType.bypass,
                replica_groups=[list(range(NC))],
                ins=[c3_d[:].opt()], outs=[G3[2:10].opt()])
            for gi in gpad_insts:
                add_dep_helper(ag3.ins, gi.ins, reason="G pads zeroed before gathers")

            # ================ LEVEL 3 (replicated) ================
            with tc.tile_pool(name="l3w", bufs=1) as wp, \
                 tc.tile_pool(name="l3st", bufs=2) as sp, \
                 tc.tile_pool(name="l3p", bufs=1) as pp, \
                 tc.tile_pool(name="l3s", bufs=2) as ss, \
                 tc.tile_pool(name="l3m", bufs=4) as sm:
                w3c1_t = wload(wp, sp, "w3c1_0")
                w3c2_t = [wload(wp, sp, "w3c2_0"), wload(wp, sp, "w3c2_1")]
                A3 = pp.tile([128, 12, 100], F32R)
                B3a = pp.tile([128, 10, 100], F32R)
                B3b = pp.tile([32, 10, 100], F32R)
                C3a = pp.tile([128, 512], F32R)
                C3b = pp.tile([32, 512], F32R)
                nc.vector.memset(B3a[:].bitcast(F32), 0.0)
                nc.vector.memset(B3b[:].bitcast(F32), 0.0)
                _r4 = nc.sync.dma_start(
                    A3[:], G3[:].rearrange("z c v -> c z v").bitcast(F32R))
                add_dep_helper(_r4.ins, ag3.ins, reason="gather before read")

                # conv1
                for (z0, nz) in ((0, 8), (2, 8)):
                    N = nz * 64
                    for (c0, co_n) in ((0, 128), (128, 32)):
                        ps = pst.tile([co_n, 512], F32, tag="ps")
                        for o, (dz, dy, dx) in enumerate(OFFSETS):
                            w = A3[:].rearrange("p z (a b) -> p z a b", b=10)
                            nc.tensor.matmul(
                                ps[:, 0:N].rearrange(
                                    "p (z a b) -> p z a b", z=nz, a=8),
                                w3c1_t[:, o, c0:c0 + co_n],
                                w[:, z0 + dz + 1:z0 + dz + 1 + nz,
                                  1 + dy:9 + dy, 1 + dx:9 + dx],
                                start=(o == 0), stop=(o == 26))
                        mt = mload(sm, "m3mul", z0 * 64, N, co_n, "m3mul")
                        B3 = B3a if c0 == 0 else B3b
                        dst = B3[:].rearrange("p z (a b) -> p z a b", b=10)
                        nc.vector.tensor_mul(
                            dst[:, z0:z0 + nz, 1:9, 1:9],
                            ps[:, 0:N].rearrange(
                                "p (z a b) -> p z a b", z=nz, a=8),
                            mt[:, 0:N].rearrange(
                                "p (z a b) -> p z a b", z=nz, a=8))

                # conv2
                for (c0, co_n) in ((0, 128), (128, 32)):
                    ps = pst.tile([co_n, 512], F32, tag="ps")
                    for o, (dz, dy, dx) in enumerate(OFFSETS):
                        for ki, B3 in enumerate((B3a, B3b)):
                            w = B3[:].rearrange("p z (a b) -> p z a b", b=10)
                            nc.tensor.matmul(
                                ps[:].rearrange("p (z a b) -> p z a b",
                                                z=8, a=8),
                                w3c2_t[ki][:, o, c0:c0 + co_n],
                                w[:, dz + 1:dz + 9, 1 + dy:9 + dy,
                                  1 + dx:9 + dx],
                                start=(o == 0 and ki == 0),
                                stop=(o == 26 and ki == 1))
                    C3 = C3a if c0 == 0 else C3b
                    if mf[3]:
                        mt = mload(sm, "mn3", 0, 512, co_n, "mn3")
                        nc.vector.tensor_add(C3[:], ps[:], mt[:])
                    else:
                        nc.scalar.copy(C3[:], ps[:])

                # pool -> P4
                for C3, P4, cn in ((C3a, P4a, 128), (C3b, P4b, 32)):
                    v = C3[:].rearrange("p (z v) -> p z v", v=64)
                    t1 = ss.tile([cn, 4, 64], F32R, tag="pool3a")
                    nc.vector.tensor_max(t1[:], v[:, 0::2, :], v[:, 1::2, :])
                    u = t1[:].rearrange("p z (a b) -> p z a b", b=8)
                    t2 = ss.tile([cn, 4, 4, 8], F32R, tag="pool3b")
                    nc.vector.tensor_max(t2[:], u[:, :, 0::2, :],
                                         u[:, :, 1::2, :])
                    dst = P4[:].rearrange("p (z a b) -> p z a b", z=6, a=6)
                    if mf[4]:
                        t3 = ss.tile([cn, 4, 4, 4], F32R, tag="pool3c")
                        nc.vector.tensor_max(t3[:], t2[:, :, :, 0::2],
                                             t2[:, :, :, 1::2])
                        mt = mload(sm, "m4p", 0, 64, cn, "m4p")
                        nc.vector.tensor_mul(
                            dst[:, 1:5, 1:5, 1:5], t3[:],
                            mt[:].rearrange("p (z a b) -> p z a b", z=4, a=4))
                    else:
                        nc.vector.tensor_max(dst[:, 1:5, 1:5, 1:5],
                                             t2[:, :, :, 0::2],
                                             t2[:, :, :, 1::2])

                if DBG:
                    nc.sync.dma_start(dbg_d["dA3"][:], A3[:].bitcast(F32))
                    nc.sync.dma_start(dbg_d["dB3a"][:], B3a[:].bitcast(F32))
                    nc.sync.dma_start(dbg_d["dC3a"][:], C3a[:].bitcast(F32))

            # ================ TAIL (levels 4-6, replicated) ================
            def tail_conv(sm, wts, ins, outs, pg, og, mode, msec, mname):
                N = og * og * og
                noff = wts[0].shape[1]
                offs = OFFSETS if noff == 27 else [(0, 0, 0)]
                for (ot, c0, co_n, padded) in outs:
                    ps = pst.tile([co_n, max(N, 8)], F32, tag="ps")
                    nmm = len(offs) * len(ins)
                    i = 0
                    for o, (dz, dy, dx) in enumerate(offs):
                        for ki, it in enumerate(ins):
                            w = it[:].rearrange("p (z a b) -> p z a b",
                                                z=pg, a=pg)
                            nc.tensor.matmul(
                                ps[:, 0:N].rearrange(
                                    "p (z a b) -> p z a b", z=og, a=og),
                                wts[ki][:, o, c0:c0 + co_n],
                                w[:, 1 + dz:1 + dz + og, 1 + dy:1 + dy + og,
                                  1 + dx:1 + dx + og],
                                start=(i == 0), stop=(i == nmm - 1))
                            i += 1
                    if padded:
                        opg = og + 2
                        dst = ot[:].rearrange("p (z a b) -> p z a b",
                                              z=opg, a=opg)[:, 1:1 + og,
                                                            1:1 + og, 1:1 + og]
                    else:
                        dst = ot[:, 0:N].rearrange("p (z a b) -> p z a b",
                                                   z=og, a=og)
                    src = ps[:, 0:N].rearrange("p (z a b) -> p z a b",
                                               z=og, a=og)
                    if mode == "copy":
                        nc.scalar.copy(dst, src)
                    else:
                        mt = mload(sm, msec, 0, N, co_n, mname)
                        mm = mt[:].rearrange("p (z a b) -> p z a b", z=og, a=og)
                        if mode == "mul":
                            nc.vector.tensor_mul(dst, src, mm)
                        else:
                            nc.vector.tensor_add(dst, src, mm)

            def tail_pool(sm, ss, cs, ps_out, g, has_mask, msec):
                go = g // 2
                for (ct, cn), (pt, _) in zip(cs, ps_out):
                    v = ct[:, 0:g * g * g].rearrange("p (z v) -> p z v",
                                                     v=g * g)
                    t1 = ss.tile([cn, go, g * g], F32, tag=f"tp{g}a")
                    nc.vector.tensor_max(t1[:], v[:, 0::2, :], v[:, 1::2, :])
                    u = t1[:].rearrange("p z (a b) -> p z a b", b=g)
                    t2 = ss.tile([cn, go, go, g], F32, tag=f"tp{g}b")
                    nc.vector.tensor_max(t2[:], u[:, :, 0::2, :],
                                         u[:, :, 1::2, :])
                    gp = go + 2
                    dst = pt[:].rearrange("p (z a b) -> p z a b", z=gp, a=gp)
                    if has_mask:
                        t3 = ss.tile([cn, go, go, go], F32, tag=f"tp{g}c")
                        nc.vector.tensor_max(t3[:], t2[:, :, :, 0::2],
                                             t2[:, :, :, 1::2])
                        mt = mload(sm, msec, 0, go * go * go, cn, f"tp{g}m")
                        nc.vector.tensor_mul(
                            dst[:, 1:1 + go, 1:1 + go, 1:1 + go], t3[:],
                            mt[:].rearrange("p (z a b) -> p z a b",
                                            z=go, a=go))
                    else:
                        nc.vector.tensor_max(
                            dst[:, 1:1 + go, 1:1 + go, 1:1 + go],
                            t2[:, :, :, 0::2], t2[:, :, :, 1::2])

            # ---- L4 ----
            with tc.tile_pool(name="l4w", bufs=1) as wp, \
                 tc.tile_pool(name="l4st", bufs=2) as sp, \
                 tc.tile_pool(name="l4p", bufs=1) as pp, \
                 tc.tile_pool(name="l4s", bufs=2) as ss, \
                 tc.tile_pool(name="l4m", bufs=2) as sm:
                w4c1_t = [wload(wp, sp, "w4c1_0", dt=F32),
                          wload(wp, sp, "w4c1_1", dt=F32)]
                w4c2_t = [wload(wp, sp, "w4c2_0", dt=F32),
                          wload(wp, sp, "w4c2_1", dt=F32)]
                B4a = pp.tile([128, 216], F32); B4b = pp.tile([64, 216], F32)
                C4a = pp.tile([128, 64], F32); C4b = pp.tile([64, 64], F32)
                nc.vector.memset(B4a[:].bitcast(F32), 0.0)
                nc.vector.memset(B4b[:].bitcast(F32), 0.0)
                tail_conv(sm, w4c1_t, [P4a, P4b],
                          [(B4a, 0, 128, True), (B4b, 128, 64, True)], 6, 4,
                          "mul" if mf[4] else "copy", "m4mul", "m4mul")
                tail_conv(sm, w4c2_t, [B4a, B4b],
                          [(C4a, 0, 128, False), (C4b, 128, 64, False)], 6, 4,
                          "add" if mf[4] else "copy", "mn4", "mn4")
                tail_pool(sm, ss, [(C4a, 128), (C4b, 64)],
                          [(P5a, 128), (P5b, 64)], 4, mf[5], "m5p")

                if DBG:
                    nc.sync.dma_start(dbg_d["dP4a"][:], P4a[:])
                    nc.sync.dma_start(dbg_d["dB4a"][:], B4a[:])
                    nc.sync.dma_start(dbg_d["dC4a"][:], C4a[:])

            # ---- L5 ----
            with tc.tile_pool(name="l5w", bufs=1) as wp, \
                 tc.tile_pool(name="l5st", bufs=2) as sp, \
                 tc.tile_pool(name="l5p", bufs=1) as pp, \
                 tc.tile_pool(name="l5s", bufs=2) as ss, \
                 tc.tile_pool(name="l5m", bufs=2) as sm:
                w5c1_t = [wload(wp, sp, "w5c1_0", dt=F32),
                          wload(wp, sp, "w5c1_1", dt=F32)]
                w5c2_t = [wload(wp, sp, "w5c2_0", dt=F32),
                          wload(wp, sp, "w5c2_1", dt=F32)]
                B5a = pp.tile([128, 64], F32); B5b = pp.tile([96, 64], F32)
                C5a = pp.tile([128, 8], F32); C5b = pp.tile([96, 8], F32)
                nc.vector.memset(B5a[:].bitcast(F32), 0.0)
                nc.vector.memset(B5b[:].bitcast(F32), 0.0)
                tail_conv(sm, w5c1_t, [P5a, P5b],
                          [(B5a, 0, 128, True), (B5b, 128, 96, True)], 4, 2,
                          "mul" if mf[5] else "copy", "m5mul", "m5mul")
                tail_conv(sm, w5c2_t, [B5a, B5b],
                          [(C5a, 0, 128, False), (C5b, 128, 96, False)], 4, 2,
                          "add" if mf[5] else "copy", "mn5", "mn5")
                tail_pool(sm, ss, [(C5a, 128), (C5b, 96)],
                          [(P6a, 128), (P6b, 96)], 2, mf[6], "m6p")

                if DBG:
                    nc.sync.dma_start(dbg_d["dP5a"][:], P5a[:])
                    nc.sync.dma_start(dbg_d["dB5a"][:], B5a[:])
                    nc.sync.dma_start(dbg_d["dP6a"][:], P6a[:])

            # ---- L6 (1^3, center tap only) ----
            with tc.tile_pool(name="l6w", bufs=1) as wp, \
                 tc.tile_pool(name="l6st", bufs=2) as sp:
                w6c1_t = [wload(wp, sp, "w6c1_0", dt=F32),
                          wload(wp, sp, "w6c1_1", dt=F32)]
                w6c2_t = [wload(wp, sp, "w6c2_0", dt=F32),
                          wload(wp, sp, "w6c2_1", dt=F32)]
                for (ot, c0) in ((X6a, 0), (X6b, 128)):
                    ps = pst.tile([128, 8], F32, tag="ps")
                    nc.tensor.matmul(ps[:, 0:1], w6c1_t[0][:, 0, c0:c0 + 128],
                                     P6a[:, 13:14], start=True, stop=False)
                    nc.tensor.matmul(ps[:, 0:1], w6c1_t[1][:, 0, c0:c0 + 128],
                                     P6b[:, 13:14], start=False, stop=True)
                    nc.vector.tensor_copy(ot[:], ps[:, 0:1])
                for i, c0 in enumerate((0, 128)):
                    ps = pst.tile([128, 8], F32, tag="ps")
                    nc.tensor.matmul(ps[:, 0:1], w6c2_t[0][:, 0, c0:c0 + 128],
                                     X6a[:], start=True, stop=False)
                    nc.tensor.matmul(ps[:, 0:1], w6c2_t[1][:, 0, c0:c0 + 128],
                                     X6b[:], start=False, stop=True)
                    nc.scalar.copy(outt[:, i:i + 1], ps[:, 0:1])
            if DBG:
                nc.sync.dma_start(dbg_d["dX6a"][:], X6a[:])
            nc.sync.dma_start(out_d[0, 0:128], outt[:, 0])
            nc.sync.dma_start(out_d[0, 128:256], outt[:, 1])

    nc.compile()
    return nc


_CACHE = {}


def kernel(features, coors, W0, W1, W2, W3, W4, W5, W6, W7, W8, W9, W10, W11,
           W12, W13):
    features = np.asarray(features, np.float32)
    coors = np.asarray(coors, np.int32)
    Ws = [np.asarray(w, np.float32) for w in
          (W0, W1, W2, W3, W4, W5, W6, W7, W8, W9, W10, W11, W12, W13)]
    in_maps, meta = build_host_inputs(features, coors, Ws)
    key = tuple(sorted(meta["mask_flags"].items()))
    if key not in _CACHE:
        _CACHE[key] = build_kernel(meta)
    nc = _CACHE[key]
    res = run_bass_kernel_spmd(nc, in_maps, core_ids=list(range(NC)))
    out = res.results[0]["out"].reshape(256)
    return out.reshape(1, 1, 1, 1, 256).astype(np.float32)


if __name__ == "__main__":
    pass


# revision 25
# speedup vs baseline: 1.0409x; 1.0409x over previous
"""Trainium2 Bass kernel for the sparse submanifold 3D CNN (nn_Net_38963943309313).

Network: 7 blocks of 2 submanifold 3x3x3 convs on a 64^3 grid, 2x2x2 sparse
max-pools between blocks, channels 3->64->...->256, output [1,1,1,1,256].

Strategy (8 NeuronCores):
 - Shard z-slabs across cores for levels 0-2 (grids 64/32/16), AllGather the
   pooled activations between levels. Levels 3-6 (grids 8/4/2/1) replicated.
 - Convs are fp32r matmuls: activations channel-major [C, z, y, x] in SBUF
   (y/x zero-padded), 27 shifted-window matmuls accumulated in PSUM.
 - Wire-format optimization (the dominant per-call cost is host->device
   upload through the PJRT tunnel): every core receives just 2 arrays -
     blob16 [DLEN+WSH] fp16: its padded dense-input z-slab [12,3,66,66]
                             followed by 1/8 of the packed conv weights,
     aux8   [AUX8TOT] uint8: the independent 0/1 mask slabs.
   Weights are AllGathered on-device (DRAM->DRAM, fp16) and upcast on load;
   all f32 mask sections (including the (m-1)*3e4 mn* evictors and the m*p
   aliases) are derived on device into a DRAM f32 scratch and broadcast-read
   from there; conv1 of block 0 is an fp16 matmul over an im2col tile built
   on-device by 9 strided DMAs per slice.
 - Instruction-count optimization (per-call executable load costs ~36us per
   static instruction): L0's conv2+pool - the largest instruction mass - runs
   as a tc.For_i hardware loop over z-pairs with register-offset APs. A0
   holds all 10 h1 slices in fp16 (slot s = rows0 h1[s], rows64 h1[s+1]);
   every in-loop matmul keeps tile_position (0,0) (base-64 moving operands
   with register APs miscompute), so the dz=+1 leftover taps read the
   primary rows0 copies of slots z+2/z+3.
 - Submanifold masking: conv1 evictions multiply by a broadcast mask; conv2
   evictions add (mask-1)*3e4 so the following max-pool ignores inactive
   voxels (activations are O(50), so 3e4 dominates); pool result is
   multiplied by the pooled mask.
"""

import sys

sys.path.insert(0, "/opt/trn_rl_repo")

import numpy as np
import concourse.bass as bass
import concourse.tile as tile
from concourse.tile import add_dep_helper
from concourse import bacc, mybir
from concourse.bass_utils import run_bass_kernel_spmd

NC = 8
GRID = 64
BIGH = 3.0e4  # fp16-representable "infinity" for mask eviction
F32 = mybir.dt.float32
F32R = mybir.dt.float32r
F16 = mybir.dt.float16
U8 = mybir.dt.uint8

OFFSETS = [(dz, dy, dx) for dz in (-1, 0, 1) for dy in (-1, 0, 1) for dx in (-1, 0, 1)]
DYDX = [(dy, dx) for dy in (-1, 0, 1) for dx in (-1, 0, 1)]


def _wlayout():
    specs = [
        ("w81", (81, 128)),
        ("w0p", (128, 9, 64)), ("w0l", (128, 9, 64)),
        ("w1p", (128, 9, 96)), ("w1l", (128, 9, 96)),
        ("w1c2_0", (96, 27, 96)),
        ("w2c1_0", (96, 27, 128)), ("w2c2_0", (128, 27, 128)),
        ("w3c1_0", (128, 27, 160)),
        ("w3c2_0", (128, 27, 160)), ("w3c2_1", (32, 27, 160)),
        ("w4c1_0", (128, 27, 192)), ("w4c1_1", (32, 27, 192)),
        ("w4c2_0", (128, 27, 192)), ("w4c2_1", (64, 27, 192)),
        ("w5c1_0", (128, 27, 224)), ("w5c1_1", (64, 27, 224)),
        ("w5c2_0", (128, 27, 224)), ("w5c2_1", (96, 27, 224)),
        ("w6c1_0", (128, 1, 256)), ("w6c1_1", (96, 1, 256)),
        ("w6c2_0", (128, 1, 256)), ("w6c2_1", (128, 1, 256)),
    ]
    off, lay = 0, {}
    for nm, sh in specs:
        lay[nm] = (off, sh)
        off += int(np.prod(sh))
    return lay, off


WLAYOUT, WTOT = _wlayout()
assert WTOT % NC == 0
WSH = WTOT // NC


def _auxlayout():
    specs = [
        ("m0mul", 10 * 4096), ("mn0", 8 * 4096),
        ("m1p", 4 * 1024), ("m1mul", 6 * 1024), ("mn1", 4 * 1024),
        ("m2p", 2 * 256), ("m2mul", 4 * 256), ("mn2", 2 * 256),
        ("m3p", 64), ("m3mul", 10 * 64), ("mn3", 8 * 64),
        ("m4p", 64), ("m4mul", 64), ("mn4", 64),
        ("m5p", 8), ("m5mul", 8), ("mn5", 8),
        ("m6p", 1),
    ]
    off, lay = 0, {}
    for nm, n in specs:
        lay[nm] = off
        off += n
    tot = ((off + 127) // 128) * 128
    return lay, tot


AUXL, AUXTOT = _auxlayout()
AUXC = AUXTOT // 128


def _aux8layout():
    # wire format: only the independent 0/1 masks; everything else (the
    # m*p aliases and the (m-1)*BIG mn* tensors) is derived on device.
    specs = [("m0mul", 10 * 4096), ("m1mul", 6 * 1024), ("m2mul", 4 * 256),
             ("m3mul", 10 * 64), ("m3p", 64), ("m45", 64), ("m5", 8), ("m6", 1)]
    off, lay = 0, {}
    for nm, n in specs:
        lay[nm] = off
        off += n
    return lay, ((off + 127) // 128) * 128


AUX8L, AUX8TOT = _aux8layout()

DOFF, DLEN = 0, 12 * 3 * 66 * 66     # dense slab inside blob16
WOFF = DLEN                          # weight shard inside blob16
BLOBTOT = DLEN + WSH


def build_host_inputs(features, coors, Ws):
    """Host-side data marshalling. Returns (in_maps, meta)."""
    z, y, x = coors[:, 0], coors[:, 1], coors[:, 2]
    dense = np.zeros((GRID, GRID, GRID, 3), np.float32)
    mask0 = np.zeros((GRID, GRID, GRID), np.float32)
    dense[z, y, x] = features  # last write wins (matches XLA CPU scatter)
    mask0[z, y, x] = 1.0

    # mask pyramid
    masks = [mask0]
    m = mask0
    for _ in range(6):
        mr = m.reshape(m.shape[0] // 2, 2, m.shape[1] // 2, 2, m.shape[2] // 2, 2)
        m = mr.max(axis=(1, 3, 5))
        masks.append(m)

    # padded dense fp16 [68, 3, 66, 66]; core k ships slab [8k, 8k+12)
    # (padded-z index = global z + 2)
    dp = np.zeros((GRID + 4, 3, GRID + 2, GRID + 2), np.float16)
    dp[2:-2, :, 1:-1, 1:-1] = dense.transpose(0, 3, 1, 2).astype(np.float16)
    dense16 = [np.ascontiguousarray(dp[8 * k:8 * k + 12]) for k in range(NC)]

    # ---- weight pack (fp16, single copy, sharded 1/8 per core) ----
    W0 = Ws[0]  # [3,3,3,3,64]
    # im2col row order (ky kx kz ci) to match the on-device xs build
    w81 = W0.transpose(1, 2, 0, 3, 4).reshape(81, 64)
    w81 = np.concatenate([w81, w81], axis=1)  # [81, 128] co-dup

    def pack_pair(W):  # [3,3,3,cin,co] -> pair [2*cin, 9, co] + left [cin, 9, co]
        cin, co = W.shape[3], W.shape[4]
        wp = np.zeros((2 * cin, 9, co), np.float32)
        wl = np.zeros((cin, 9, co), np.float32)
        for j, (dy, dx) in enumerate(DYDX):
            wp[0:cin, j] = W[0, dy + 1, dx + 1]
            wp[cin:2 * cin, j] = W[1, dy + 1, dx + 1]
            wl[:, j] = W[2, dy + 1, dx + 1]
        return wp, wl

    w0p, w0l = pack_pair(Ws[1])
    w1p, w1l = pack_pair(Ws[2])
    w0l = np.concatenate([w0l, w0l], axis=0)  # [128, 9, 64]
    w1l = np.concatenate([w1l, w1l], axis=0)  # [128, 9, 96]

    def pack_generic(W):  # -> list of [kchunk, 27, co]
        cin, co = W.shape[3], W.shape[4]
        wf = W.reshape(27, cin, co)
        return [np.ascontiguousarray(wf[:, k0:k0 + min(128, cin - k0), :]
                                     .transpose(1, 0, 2))
                for k0 in range(0, cin, 128)]

    wtensors = {"w81": w81, "w0p": w0p, "w0l": w0l, "w1p": w1p, "w1l": w1l}
    for li, wi in [("w1c2", 3), ("w2c1", 4), ("w2c2", 5), ("w3c1", 6),
                   ("w3c2", 7), ("w4c1", 8), ("w4c2", 9), ("w5c1", 10),
                   ("w5c2", 11)]:
        for ci, arr in enumerate(pack_generic(Ws[wi])):
            wtensors[f"{li}_{ci}"] = arr
    for li, wi in [("w6c1", 12), ("w6c2", 13)]:
        W = Ws[wi]
        cin = W.shape[3]
        wc = W[1, 1, 1]  # [cin, co]
        for ci, k0 in enumerate(range(0, cin, 128)):
            wtensors[f"{li}_{ci}"] = np.ascontiguousarray(
                wc[k0:k0 + min(128, cin - k0)][:, None, :])

    W16 = np.zeros(WTOT, np.float16)
    for nm, (off, sh) in WLAYOUT.items():
        arr = wtensors[nm]
        assert tuple(arr.shape) == tuple(sh), (nm, arr.shape, sh)
        n = int(np.prod(sh))
        W16[off:off + n] = arr.astype(np.float16).reshape(-1)
    wshards = [np.ascontiguousarray(W16[k * WSH:(k + 1) * WSH])
               for k in range(NC)]

    # ---- aux mask pack (uint8 0/1 wire format, per core) ----
    def slab_mask(mask, z0, nsl):
        D2 = mask.shape[1] * mask.shape[2]
        out = np.zeros((nsl, D2), np.float32)
        for i in range(nsl):
            zg = z0 + i
            if 0 <= zg < mask.shape[0]:
                out[i] = mask[zg].reshape(-1)
        return out

    def mkaux8(k):
        parts = [
            slab_mask(masks[0], 8 * k - 1, 10),   # m0mul
            slab_mask(masks[1], 4 * k - 1, 6),    # m1mul
            slab_mask(masks[2], 2 * k - 1, 4),    # m2mul
            slab_mask(masks[3], -1, 10),          # m3mul
            slab_mask(masks[3], k, 1),            # m3p
            masks[4].reshape(1, -1),              # m45
            masks[5].reshape(1, -1),              # m5
            masks[6].reshape(1, -1),              # m6
        ]
        flat = np.concatenate([p.reshape(-1) for p in parts])
        out = np.zeros(AUX8TOT, np.uint8)
        out[:flat.size] = flat.astype(np.uint8)
        return out

    meta = {
        "mask_flags": {
            lvl: not np.all(masks[lvl] == 1.0) for lvl in range(1, 7)
        },
    }

    in_maps = [{"blob16": np.concatenate([dense16[k].reshape(-1), wshards[k]]),
                "aux8": mkaux8(k)} for k in range(NC)]
    return in_maps, meta


def build_kernel(meta):
    import contextlib
    nc = bacc.Bacc("TRN2", target_bir_lowering=False, debug=False, num_devices=NC)
    mf = meta["mask_flags"]

    blob16_d = nc.dram_tensor("blob16", [BLOBTOT], F16, kind="ExternalInput")
    aux8_d = nc.dram_tensor("aux8", [AUX8TOT], U8, kind="ExternalInput")
    out_d = nc.dram_tensor("out", [1, 256], F32, kind="ExternalOutput")

    import os as _os
    DBG = bool(_os.environ.get("K_DEBUG"))
    dbg_d = {}
    if DBG:
        for nm, sh in [("dP0", (64, 4, 1156)), ("dA1", (128, 8, 1156)),
                       ("dB1", (96, 6, 1156)), ("dC1", (96, 4, 1024)),
                       ("dP1", (96, 2, 324)), ("dA2", (96, 6, 324)),
                       ("dB2", (128, 4, 324)), ("dC2", (128, 2, 256)),
                       ("dP2", (128, 1, 100)), ("dA3", (128, 12, 100)),
                       ("dB3a", (128, 10, 100)), ("dC3a", (128, 512)),
                       ("dP4a", (128, 216)), ("dB4a", (128, 216)),
                       ("dC4a", (128, 64)), ("dP5a", (128, 64)),
                       ("dB5a", (128, 64)), ("dP6a", (128, 27)),
                       ("dX6a", (128, 1)), ("dC0", (64, 2, 4096)),
                       ("dXS", (81, 4096))]:
            dbg_d[nm] = nc.dram_tensor(nm, list(sh), F32, kind="ExternalOutput")

    with tile.TileContext(nc) as tc:
        ctx = contextlib.ExitStack()
        with ctx:
            pst = ctx.enter_context(tc.tile_pool(name="ps", bufs=4, space="PSUM"))
            drm = ctx.enter_context(tc.tile_pool(name="dram", bufs=1, space="DRAM"))
            glob = ctx.enter_context(tc.tile_pool(name="glob", bufs=1))

            pid = nc.sync.partition_id()

            # ---------- weight shard AllGather + aux mask upcast ----------
            WST = drm.tile([WSH], F16)
            GW = drm.tile([WTOT], F16)
            AUXF = drm.tile([AUXTOT], F32)

            cpw = nc.sync.dma_start(WST[:], blob16_d[WOFF:WOFF + WSH])
            agw = nc.gpsimd.collective_compute(
                "AllGather", mybir.AluOpType.bypass,
                replica_groups=[list(range(NC))],
                ins=[WST[:].opt()], outs=[GW[:].opt()])
            add_dep_helper(agw.ins, cpw.ins, reason="stage shard before gather")

            # dense slab view inside the blob
            DNS = blob16_d[DOFF:DOFF + DLEN].rearrange(
                "(z c a b) -> z c a b", c=3, a=66, b=66)

            # derive all f32 mask sections (and mn* = (m-1)*BIGH) from the
            # uint8 wire masks into the AUXF DRAM scratch
            AUXWB = {}
            with tc.tile_pool(name="prep", bufs=2) as prep:
                def aux_sec(dst, src_off, n, affine):
                    p = 128 if n % 128 == 0 else n
                    c = n // p
                    tu = prep.tile([p, c], U8, tag="a8")
                    nc.sync.dma_start(
                        tu[:], aux8_d[src_off:src_off + n]
                        .rearrange("(p c) -> p c", c=c))
                    tf = prep.tile([p, c], F32, tag="af")
                    if affine:
                        tc_ = prep.tile([p, c], F32, tag="ac")
                        nc.vector.tensor_copy(tc_[:], tu[:])
                        nc.vector.tensor_scalar(tf[:], tc_[:], BIGH, -BIGH,
                                                mybir.AluOpType.mult,
                                                mybir.AluOpType.add)
                    else:
                        nc.vector.tensor_copy(tf[:], tu[:])
                    AUXWB[dst] = nc.sync.dma_start(
                        AUXF[AUXL[dst]:AUXL[dst] + n]
                        .rearrange("(p c) -> p c", c=c), tf[:])

                aux_sec("m0mul", AUX8L["m0mul"], 40960, False)
                aux_sec("mn0", AUX8L["m0mul"] + 4096, 32768, True)
                aux_sec("m1p", AUX8L["m1mul"] + 1024, 4096, False)
                aux_sec("m1mul", AUX8L["m1mul"], 6144, False)
                aux_sec("mn1", AUX8L["m1mul"] + 1024, 4096, True)
                aux_sec("m2p", AUX8L["m2mul"] + 256, 512, False)
                aux_sec("m2mul", AUX8L["m2mul"], 1024, False)
                aux_sec("mn2", AUX8L["m2mul"] + 256, 512, True)
                aux_sec("m3p", AUX8L["m3p"], 64, False)
                aux_sec("m3mul", AUX8L["m3mul"], 640, False)
                aux_sec("mn3", AUX8L["m3mul"] + 64, 512, True)
                aux_sec("m4p", AUX8L["m45"], 64, False)
                aux_sec("m4mul", AUX8L["m45"], 64, False)
                aux_sec("mn4", AUX8L["m45"], 64, True)
                aux_sec("m5p", AUX8L["m5"], 8, False)
                aux_sec("m5mul", AUX8L["m5"], 8, False)
                aux_sec("mn5", AUX8L["m5"], 8, True)
                aux_sec("m6p", AUX8L["m6"], 1, False)

            def mload(sm, sec, rel, n, parts, tag):
                mt = sm.tile([parts, n], F32, tag=tag)
                off = AUXL[sec] + rel
                r = nc.sync.dma_start(
                    mt[:], AUXF[off:off + n].unsqueeze(0).to_broadcast((parts, n)))
                add_dep_helper(r.ins, AUXWB[sec].ins, reason="aux masks written")
                return mt

            def wload(pool, spool, nm, dt=F32R, name=None):
                off, sh = WLAYOUT[nm]
                p, c = sh[0], int(np.prod(sh[1:]))
                t16 = spool.tile([p, c], F16, tag="wst", name=f"wst_{nm}")
                r = nc.sync.dma_start(
                    t16[:], GW[off:off + p * c].rearrange("(p c) -> p c", c=c))
                add_dep_helper(r.ins, agw.ins, reason="weights gathered")
                t = pool.tile(list(sh), dt, name=name or f"sb_{nm}")
                dv = t[:]
                if len(sh) == 3:
                    dv = dv.rearrange("p a b -> p (a b)")
                nc.vector.tensor_copy(dv, t16[:])
                return t

            def wload16raw(pool, nm):
                off, sh = WLAYOUT[nm]
                p, c = sh[0], int(np.prod(sh[1:]))
                t = pool.tile(list(sh), F16, name=f"sb_{nm}")
                dv = t[:]
                if len(sh) == 3:
                    dv = dv.rearrange("p a b -> p (a b)")
                r = nc.sync.dma_start(
                    dv, GW[off:off + p * c].rearrange("(p c) -> p c", c=c))
                add_dep_helper(r.ins, agw.ins, reason="weights gathered")
                return t

            # zero tile for G-pad zeroing
            zt = glob.tile([128, 1156], F32)
            nc.vector.memset(zt[:], 0.0)

            # DRAM gather buffers
            c1_d = drm.tile([4, 64, 1156], F32)
            G1 = drm.tile([36, 64, 1156], F32)
            c2_d = drm.tile([2, 96, 324], F32)
            G2 = drm.tile([20, 96, 324], F32)
            c3_d = drm.tile([1, 128, 100], F32)
            G3 = drm.tile([12, 128, 100], F32)
            gpad_insts = []
            for G, csz, npad in ((G1, (64, 1156), 2), (G2, (96, 324), 2),
                                 (G3, (128, 100), 2)):
                n = G.shape[0]
                for s in list(range(npad)) + list(range(n - npad, n)):
                    gpad_insts.append(
                        nc.sync.dma_start(G[s], zt[0:csz[0], 0:csz[1]]))

            # persistent tail tensors
            P4a = glob.tile([128, 216], F32); P4b = glob.tile([32, 216], F32)
            P5a = glob.tile([128, 64], F32); P5b = glob.tile([64, 64], F32)
            P6a = glob.tile([128, 27], F32); P6b = glob.tile([96, 27], F32)
            X6a = glob.tile([128, 1], F32); X6b = glob.tile([128, 1], F32)
            outt = glob.tile([128, 2], F32)
            for t in (P4a, P4b, P5a, P5b, P6a, P6b):
                nc.vector.memset(t[:].bitcast(F32), 0.0)

            # ================ LEVEL 0 ================
            with tc.tile_pool(name="l0w", bufs=1) as wp, \
                 tc.tile_pool(name="l0st", bufs=2) as sp, \
                 tc.tile_pool(name="l0p", bufs=1) as pp, \
                 tc.tile_pool(name="l0s", bufs=2) as ss, \
                 tc.tile_pool(name="l0m", bufs=4) as sm:
                w81_t = wload16raw(wp, "w81")
                w0p_t = wload16raw(wp, "w0p")   # [128, 9, 64] f16
                w0l_t = wload16raw(wp, "w0l")

                # A0: 10 h1 slices, fp16, slot s = (rows0: h1[s], rows64: h1[s+1])
                A0 = pp.tile([128, 10, 4356], F16)
                C0 = pp.tile([64, 2, 4096], F32R)
                P0 = pp.tile([64, 4, 1156], F32R)
                nc.vector.memset(A0[:], 0.0)
                nc.vector.memset(P0[:].bitcast(F32), 0.0)

                # conv1: hardware loop over slices (prologue sl=0 writes only
                # the rows0 copy). DNSr = dense slab as [(z c)=36, 66, 66].
                DNSr = blob16_d[DOFF:DOFF + DLEN].rearrange(
                    "(p a b) -> p a b", a=66, b=66)
                A0l0 = A0[0:64, :, :].rearrange("p s (a b) -> p (s a) b", b=66)
                A0h0 = A0[64:128, :, :].rearrange("p s (a b) -> p (s a) b", b=66)

                with tc.For_i(0, 10, 1) as slv:
                    # on-device im2col: xs rows = 9*(ky*3+kx) + (kz*3+ci)
                    xs = ss.tile([81, 4096], F16, tag="x1s")
                    for jj, (dy, dx) in enumerate(DYDX):
                        nc.sync.dma_start(
                            xs[9 * jj:9 * jj + 9].rearrange(
                                "p (a b) -> p a b", b=64),
                            DNSr[bass.ds(slv * 3, 9), 1 + dy:65 + dy,
                                 1 + dx:65 + dx])
                    for chunk in range(8):
                        ps = pst.tile([128, 512], F32, tag="ps")
                        nc.tensor.matmul(ps[:], w81_t[:],
                                         xs[:, chunk * 512:chunk * 512 + 512],
                                         start=True, stop=True)
                        mt = sm.tile([64, 512], F32, tag="m0")
                        r = nc.sync.dma_start(
                            mt[:],
                            AUXF[bass.ds(AUXL["m0mul"] + slv * 4096 + chunk * 512,
                                         512)]
                            .unsqueeze(0).to_broadcast((64, 512)))
                        add_dep_helper(r.ins, AUXWB["m0mul"].ins,
                                       reason="aux masks written")
                        yb = chunk * 8
                        nc.vector.tensor_mul(
                            A0l0[:, bass.ds(slv * 66 + yb + 1, 8), 1:65],
                            ps[0:64, :].rearrange("p (a b) -> p a b", b=64),
                            mt[:].rearrange("p (a b) -> p a b", b=64))
                # rows64 duplicates (already masked): slot s rows64 = h1[s+1]
                nc.vector.tensor_copy(
                    A0[64:128, 0:9, :].rearrange("p s v -> p (s v)"),
                    A0[0:64, 1:10, :].rearrange("p s v -> p (s v)"))

                # conv2 + pool: hardware loop over z-pairs (zv = 0,2,4,6).
                # A0r rows = slot*66 + y; out zv pair=slot zv, out zv+1
                # pair=slot zv+1; dz=+1 leftovers h1[zv+2]/h1[zv+3] read from
                # the primary copies (rows 0:64 of slots zv+2 / zv+3) so every
                # register-offset matmul stays at tile_position (0, 0) —
                # base-64 moving operands with register APs produce NaNs.
                A0r = A0[:].rearrange("p s (a b) -> p (s a) b", b=66)
                A0l = A0[0:64, :, :].rearrange("p s (a b) -> p (s a) b", b=66)
                P0r = P0[:].rearrange("p s (a b) -> p (s a) b", b=34)
                with tc.For_i(0, 8, 2) as zv:
                    for chunk in range(8):
                        yb = chunk * 8
                        psA = pst.tile([64, 512], F32, tag="ps")
                        psB = pst.tile([64, 512], F32, tag="ps")
                        for j, (dy, dx) in enumerate(DYDX):
                            first, last = (j == 0), (j == 8)
                            xsl = slice(1 + dx, 65 + dx)
                            vA = psA[:].rearrange("p (a b) -> p a b", b=64)
                            vB = psB[:].rearrange("p (a b) -> p a b", b=64)
                            rbase = zv * 66 + (yb + 1 + dy)
                            nc.tensor.matmul(
                                vA, w0p_t[:, j, :],
                                A0r[:, bass.ds(rbase, 8), xsl],
                                start=first, stop=False,
                                tile_position=(0, 0))
                            nc.tensor.matmul(
                                vB, w0p_t[:, j, :],
                                A0r[:, bass.ds(rbase + 66, 8), xsl],
                                start=first, stop=False,
                                tile_position=(0, 0))
                            nc.tensor.matmul(
                                vA, w0l_t[0:64, j, :],
                                A0l[:, bass.ds(rbase + 132, 8), xsl],
                                start=False, stop=last,
                                tile_position=(0, 0))
                            nc.tensor.matmul(
                                vB, w0l_t[0:64, j, :],
                                A0l[:, bass.ds(rbase + 198, 8), xsl],
                                start=False, stop=last,
                                tile_position=(0, 0))
                        for ps_, h in ((psA, 0), (psB, 1)):
                            mt = sm.tile([64, 512], F32, tag="mn0")
                            r = nc.sync.dma_start(
                                mt[:],
                                AUXF[bass.ds(
                                    AUXL["mn0"] + zv * 4096 + h * 4096 + yb * 64,
                                    512)]
                                .unsqueeze(0).to_broadcast((64, 512)))
                            add_dep_helper(r.ins, AUXWB["mn0"].ins,
                                           reason="aux masks written")
                            nc.vector.tensor_add(
                                C0[:, h, yb * 64:yb * 64 + 512], ps_[:], mt[:])
                    # pool z-pair -> P0 slot zv/2
                    nc.vector.tensor_max(C0[:, 0, :], C0[:, 0, :], C0[:, 1, :])
                    v = C0[:, 0, :].rearrange("p (a b) -> p a b", b=64)
                    t2 = ss.tile([64, 32, 64], F32R, tag="pool0b", bufs=1)
                    nc.vector.tensor_max(t2[:], v[:, 0::2, :], v[:, 1::2, :])
                    t3 = ss.tile([64, 32, 32], F32R, tag="pool0c", bufs=1)
                    nc.vector.tensor_max(t3[:], t2[:, :, 0::2], t2[:, :, 1::2])
                    mtp = sm.tile([64, 1024], F32, tag="m1p")
                    rp = nc.sync.dma_start(
                        mtp[:],
                        AUXF[bass.ds(AUXL["m1p"] + zv * 512, 1024)]
                        .unsqueeze(0).to_broadcast((64, 1024)))
                    add_dep_helper(rp.ins, AUXWB["m1p"].ins,
                                   reason="aux masks written")
                    nc.vector.tensor_mul(
                        P0r[:, bass.ds(zv * 17 + 1, 32), 1:33],
                        t3[:],
                        mtp[:].rearrange("p (a b) -> p a b", b=32))

                if DBG:
                    nc.sync.dma_start(dbg_d["dP0"][:], P0[:].bitcast(F32))
                    nc.sync.dma_start(dbg_d["dC0"][:], C0[:].bitcast(F32))
                nc.sync.dma_start(c1_d[:].rearrange("z c v -> c z v"),
                                  P0[:].bitcast(F32))

            # ---- AllGather L0 -> L1 ----
            ag1 = nc.gpsimd.collective_compute(
                "AllGather", mybir.AluOpType.bypass,
                replica_groups=[list(range(NC))],
                ins=[c1_d[:].opt()], outs=[G1[2:34].opt()])
            for gi in gpad_insts:
                add_dep_helper(ag1.ins, gi.ins, reason="G pads zeroed before gathers")

            # ================ LEVEL 1 ================
            with tc.tile_pool(name="l1w", bufs=1) as wp, \
                 tc.tile_pool(name="l1st", bufs=2) as sp, \
                 tc.tile_pool(name="l1p", bufs=1) as pp, \
                 tc.tile_pool(name="l1s", bufs=2) as ss, \
                 tc.tile_pool(name="l1m", bufs=4) as sm:
                w1p_t = wload(wp, sp, "w1p")
                w1l_t = wload(wp, sp, "w1l")
                w1c2_t = wload(wp, sp, "w1c2_0")

                A1 = pp.tile([128, 8, 1156], F32R)
                B1 = pp.tile([96, 6, 1156], F32R)
                C1 = pp.tile([96, 4, 1024], F32R)
                P1 = pp.tile([96, 2, 324], F32R)
                nc.vector.memset(B1[:].bitcast(F32), 0.0)
                nc.vector.memset(P1[:].bitcast(F32), 0.0)
                _r1 = nc.sync.dma_start(
                    A1[0:64, :, :],
                    G1[bass.ds(pid * 4, 8)].rearrange("z c v -> c z v").bitcast(F32R))
                _r2 = nc.sync.dma_start(
                    A1[64:128, 0:7, :],
                    G1[bass.ds(pid * 4 + 1, 7)].rearrange("z c v -> c z v").bitcast(F32R))
                add_dep_helper(_r1.ins, ag1.ins, reason="gather before dynamic read")
                add_dep_helper(_r2.ins, ag1.ins, reason="gather before dynamic read")

                def l1_conv1(sl):
                    mt = mload(sm, "m1mul", sl * 1024, 1024, 96, "m1mul")
                    pss = [pst.tile([96, 512], F32, tag="ps",
                                    name=f"ps_l1_{sl}_{_c}") for _c in range(2)]
                    wA = A1[:, sl, :].rearrange("p (a b) -> p a b", b=34)
                    wB = A1[64:128, sl + 1, :].rearrange("p (a b) -> p a b", b=34)
                    wC = A1[0:64, sl + 2, :].rearrange("p (a b) -> p a b", b=34)
                    for j, (dy, dx) in enumerate(DYDX):
                        xsl = slice(1 + dx, 33 + dx)
                        for chunk in range(2):
                            yb = chunk * 16
                            ys = slice(yb + 1 + dy, yb + 17 + dy)
                            nc.tensor.matmul(
                                pss[chunk][:].rearrange("p (a b) -> p a b", b=32),
                                w1p_t[:, j, :], wA[:, ys, xsl],
                                start=(j == 0), stop=False)
                        ys0 = slice(1 + dy, 17 + dy)
                        ys1 = slice(17 + dy, 33 + dy)
                        nc.tensor.matmul(
                            pss[0][:].rearrange("p (a b) -> p a b", b=32),
                            w1l_t[64:128, j, :], wB[:, ys0, xsl],
                            start=False, stop=(j == 8))
                        nc.tensor.matmul(
                            pss[1][:].rearrange("p (a b) -> p a b", b=32),
                            w1l_t[0:64, j, :], wC[:, ys1, xsl],
                            start=False, stop=(j == 8))
                    for chunk in range(2):
                        yb = chunk * 16
                        dst = B1[:, sl, :].rearrange("p (a b) -> p a b", b=34)
                        nc.vector.tensor_mul(
                            dst[:, yb + 1:yb + 17, 1:33],
                            pss[chunk][:].rearrange("p (a b) -> p a b", b=32),
                            mt[:, yb * 32:yb * 32 + 512].rearrange(
                                "p (a b) -> p a b", b=32))

                def l1_conv2(sl):
                    mt = mload(sm, "mn1", sl * 1024, 1024, 96, "mn1")
                    for chunk in range(2):
                        yb = chunk * 16
                        ps = pst.tile([96, 512], F32, tag="ps")
                        for o, (dz, dy, dx) in enumerate(OFFSETS):
                            w = B1[:, sl + 1 + dz, :].rearrange(
                                "p (a b) -> p a b", b=34)
                            nc.tensor.matmul(
                                ps[:].rearrange("p (a b) -> p a b", b=32),
                                w1c2_t[:, o, :],
                                w[:, yb + 1 + dy:yb + 17 + dy, 1 + dx:33 + dx],
                                start=(o == 0), stop=(o == 26))
                        nc.vector.tensor_add(C1[:, sl, yb * 32:yb * 32 + 512],
                                             ps[:],
                                             mt[:, yb * 32:yb * 32 + 512])

                def l1_pool(zz):
                    zp = zz // 2
                    nc.vector.tensor_max(C1[:, zz, :], C1[:, zz, :], C1[:, zz + 1, :])
                    v = C1[:, zz, :].rearrange("p (a b) -> p a b", b=32)
                    t2 = ss.tile([96, 16, 32], F32R, tag="pool1b")
                    nc.vector.tensor_max(t2[:], v[:, 0::2, :], v[:, 1::2, :])
                    t3 = ss.tile([96, 16, 16], F32R, tag="pool1c")
                    nc.vector.tensor_max(t3[:], t2[:, :, 0::2], t2[:, :, 1::2])
                    mt = mload(sm, "m2p", zp * 256, 256, 96, "m2p")
                    dst = P1[:, zp, :].rearrange("p (a b) -> p a b", b=18)
                    nc.vector.tensor_mul(
                        dst[:, 1:17, 1:17], t3[:],
                        mt[:].rearrange("p (a b) -> p a b", b=16))

                for sl in range(6):
                    l1_conv1(sl)
                    if sl >= 2:
                        l1_conv2(sl - 2)
                        if sl >= 3 and (sl - 3) % 2 == 0:
                            l1_pool(sl - 3)

                if DBG:
                    nc.sync.dma_start(dbg_d["dA1"][:], A1[:].bitcast(F32))
                    nc.sync.dma_start(dbg_d["dB1"][:], B1[:].bitcast(F32))
                    nc.sync.dma_start(dbg_d["dC1"][:], C1[:].bitcast(F32))
                    nc.sync.dma_start(dbg_d["dP1"][:], P1[:].bitcast(F32))
                nc.sync.dma_start(c2_d[:].rearrange("z c v -> c z v"),
                                  P1[:].bitcast(F32))

            # ---- AllGather L1 -> L2 ----
            ag2 = nc.gpsimd.collective_compute(
                "AllGather", mybir.AluOpType.bypass,
                replica_groups=[list(range(NC))],
                ins=[c2_d[:].opt()], outs=[G2[2:18].opt()])
            for gi in gpad_insts:
                add_dep_helper(ag2.ins, gi.ins, reason="G pads zeroed before gathers")

            # ================ LEVEL 2 ================
            with tc.tile_pool(name="l2w", bufs=1) as wp, \
                 tc.tile_pool(name="l2st", bufs=2) as sp, \
                 tc.tile_pool(name="l2p", bufs=1) as pp, \
                 tc.tile_pool(name="l2s", bufs=2) as ss, \
                 tc.tile_pool(name="l2m", bufs=4) as sm:
                w2c1_t = wload(wp, sp, "w2c1_0")
                w2c2_t = wload(wp, sp, "w2c2_0")
                A2 = pp.tile([96, 6, 324], F32R)
                B2 = pp.tile([128, 4, 324], F32R)
                C2 = pp.tile([128, 2, 256], F32R)
                P2 = pp.tile([128, 1, 100], F32R)
                nc.vector.memset(B2[:].bitcast(F32), 0.0)
                nc.vector.memset(P2[:].bitcast(F32), 0.0)
                _r3 = nc.sync.dma_start(
                    A2[:], G2[bass.ds(pid * 2, 6)].rearrange("z c v -> c z v").bitcast(F32R))
                add_dep_helper(_r3.ins, ag2.ins, reason="gather before dynamic read")

                for s0 in (0, 2):
                    ps = pst.tile([128, 512], F32, tag="ps")
                    for o, (dz, dy, dx) in enumerate(OFFSETS):
                        w = A2[:].rearrange("p z (a b) -> p z a b", b=18)
                        nc.tensor.matmul(
                            ps[:].rearrange("p (z a b) -> p z a b", z=2, a=16),
                            w2c1_t[:, o, :],
                            w[:, s0 + dz + 1:s0 + dz + 3,
                              1 + dy:17 + dy, 1 + dx:17 + dx],
                            start=(o == 0), stop=(o == 26))
                    mt = mload(sm, "m2mul", s0 * 256, 512, 128, "m2mul")
                    dst = B2[:].rearrange("p z (a b) -> p z a b", b=18)
                    nc.vector.tensor_mul(
                        dst[:, s0:s0 + 2, 1:17, 1:17],
                        ps[:].rearrange("p (z a b) -> p z a b", z=2, a=16),
                        mt[:].rearrange("p (z a b) -> p z a b", z=2, a=16))

                ps = pst.tile([128, 512], F32, tag="ps")
                for o, (dz, dy, dx) in enumerate(OFFSETS):
                    w = B2[:].rearrange("p z (a b) -> p z a b", b=18)
                    nc.tensor.matmul(
                        ps[:].rearrange("p (z a b) -> p z a b", z=2, a=16),
                        w2c2_t[:, o, :],
                        w[:, dz + 1:dz + 3, 1 + dy:17 + dy, 1 + dx:17 + dx],
                        start=(o == 0), stop=(o == 26))
                if mf[2]:
                    mt = mload(sm, "mn2", 0, 512, 128, "mn2")
                    nc.vector.tensor_add(C2[:].rearrange("p a b -> p (a b)"),
                                         ps[:], mt[:])
                else:
                    nc.scalar.copy(C2[:].rearrange("p a b -> p (a b)"), ps[:])

                # L2 pool
                nc.vector.tensor_max(C2[:, 0, :], C2[:, 0, :], C2[:, 1, :])
                v = C2[:, 0, :].rearrange("p (a b) -> p a b", b=16)
                t2 = ss.tile([128, 8, 16], F32R, tag="pool2b")
                nc.vector.tensor_max(t2[:], v[:, 0::2, :], v[:, 1::2, :])
                dst = P2[:, 0, :].rearrange("p (a b) -> p a b", b=10)
                if mf[3]:
                    t3 = ss.tile([128, 8, 8], F32R, tag="pool2c")
                    nc.vector.tensor_max(t3[:], t2[:, :, 0::2], t2[:, :, 1::2])
                    mt = mload(sm, "m3p", 0, 64, 128, "m3p")
                    nc.vector.tensor_mul(
                        dst[:, 1:9, 1:9], t3[:],
                        mt[:].rearrange("p (a b) -> p a b", b=8))
                else:
                    nc.vector.tensor_max(dst[:, 1:9, 1:9],
                                         t2[:, :, 0::2], t2[:, :, 1::2])

                if DBG:
                    nc.sync.dma_start(dbg_d["dA2"][:], A2[:].bitcast(F32))
                    nc.sync.dma_start(dbg_d["dB2"][:], B2[:].bitcast(F32))
                    nc.sync.dma_start(dbg_d["dC2"][:], C2[:].bitcast(F32))
                    nc.sync.dma_start(dbg_d["dP2"][:], P2[:].bitcast(F32))
                nc.sync.dma_start(c3_d[:].rearrange("z c v -> c z v"),
                                  P2[:].bitcast(F32))

            # ---- AllGather L2 -> L3 ----
            ag3 = nc.gpsimd.collective_compute(
                "AllGather", mybir.AluOpType.bypass,
                replica_groups=[list(range(NC))],
                ins=[c3_d[:].opt()], outs=[G3[2:10].opt()])
            for gi in gpad_insts:
                add_dep_helper(ag3.ins, gi.ins, reason="G pads zeroed before gathers")

            # ================ LEVEL 3 (replicated) ================
            with tc.tile_pool(name="l3w", bufs=1) as wp, \
                 tc.tile_pool(name="l3st", bufs=2) as sp, \
                 tc.tile_pool(name="l3p", bufs=1) as pp, \
                 tc.tile_pool(name="l3s", bufs=2) as ss, \
                 tc.tile_pool(name="l3m", bufs=4) as sm:
                w3c1_t = wload(wp, sp, "w3c1_0")
                w3c2_t = [wload(wp, sp, "w3c2_0"), wload(wp, sp, "w3c2_1")]
                A3 = pp.tile([128, 12, 100], F32R)
                B3a = pp.tile([128, 10, 100], F32R)
                B3b = pp.tile([32, 10, 100], F32R)
                C3a = pp.tile([128, 512], F32R)
                C3b = pp.tile([32, 512], F32R)
                nc.vector.memset(B3a[:].bitcast(F32), 0.0)
                nc.vector.memset(B3b[:].bitcast(F32), 0.0)
                _r4 = nc.sync.dma_start(
                    A3[:], G3[:].rearrange("z c v -> c z v").bitcast(F32R))
                add_dep_helper(_r4.ins, ag3.ins, reason="gather before read")

                # conv1
                for (z0, nz) in ((0, 8), (2, 8)):
                    N = nz * 64
                    for (c0, co_n) in ((0, 128), (128, 32)):
                        ps = pst.tile([co_n, 512], F32, tag="ps")
                        for o, (dz, dy, dx) in enumerate(OFFSETS):
                            w = A3[:].rearrange("p z (a b) -> p z a b", b=10)
                            nc.tensor.matmul(
                                ps[:, 0:N].rearrange(
                                    "p (z a b) -> p z a b", z=nz, a=8),
                                w3c1_t[:, o, c0:c0 + co_n],
                                w[:, z0 + dz + 1:z0 + dz + 1 + nz,
                                  1 + dy:9 + dy, 1 + dx:9 + dx],
                                start=(o == 0), stop=(o == 26))
                        mt = mload(sm, "m3mul", z0 * 64, N, co_n, "m3mul")
                        B3 = B3a if c0 == 0 else B3b
                        dst = B3[:].rearrange("p z (a b) -> p z a b", b=10)
                        nc.vector.tensor_mul(
                            dst[:, z0:z0 + nz, 1:9, 1:9],
                            ps[:, 0:N].rearrange(
                                "p (z a b) -> p z a b", z=nz, a=8),
                            mt[:, 0:N].rearrange(
                                "p (z a b) -> p z a b", z=nz, a=8))

                # conv2
                for (c0, co_n) in ((0, 128), (128, 32)):
                    ps = pst.tile([co_n, 512], F32, tag="ps")
                    for o, (dz, dy, dx) in enumerate(OFFSETS):
                        for ki, B3 in enumerate((B3a, B3b)):
                            w = B3[:].rearrange("p z (a b) -> p z a b", b=10)
                            nc.tensor.matmul(
                                ps[:].rearrange("p (z a b) -> p z a b",
                                                z=8, a=8),
                                w3c2_t[ki][:, o, c0:c0 + co_n],
                                w[:, dz + 1:dz + 9, 1 + dy:9 + dy,
                                  1 + dx:9 + dx],
                                start=(o == 0 and ki == 0),
                                stop=(o == 26 and ki == 1))
                    C3 = C3a if c0 == 0 else C3b
                    if mf[3]:
                        mt = mload(sm, "mn3", 0, 512, co_n, "mn3")
                        nc.vector.tensor_add(C3[:], ps[:], mt[:])
                    else:
                        nc.scalar.copy(C3[:], ps[:])

                # pool -> P4
                for C3, P4, cn in ((C3a, P4a, 128), (C3b, P4b, 32)):
                    v = C3[:].rearrange("p (z v) -> p z v", v=64)
                    t1 = ss.tile([cn, 4, 64], F32R, tag="pool3a")
                    nc.vector.tensor_max(t1[:], v[:, 0::2, :], v[:, 1::2, :])
                    u = t1[:].rearrange("p z (a b) -> p z a b", b=8)
                    t2 = ss.tile([cn, 4, 4, 8], F32R, tag="pool3b")
                    nc.vector.tensor_max(t2[:], u[:, :, 0::2, :],
                                         u[:, :, 1::2, :])
                    dst = P4[:].rearrange("p (z a b) -> p z a b", z=6, a=6)
                    if mf[4]:
                        t3 = ss.tile([cn, 4, 4, 4], F32R, tag="pool3c")
                        nc.vector.tensor_max(t3[:], t2[:, :, :, 0::2],
                                             t2[:, :, :, 1::2])
                        mt = mload(sm, "m4p", 0, 64, cn, "m4p")
                        nc.vector.tensor_mul(
                            dst[:, 1:5, 1:5, 1:5], t3[:],
                            mt[:].rearrange("p (z a b) -> p z a b", z=4, a=4))
                    else:
                        nc.vector.tensor_max(dst[:, 1:5, 1:5, 1:5],
                                             t2[:, :, :, 0::2],
                                             t2[:, :, :, 1::2])

                if DBG:
                    nc.sync.dma_start(dbg_d["dA3"][:], A3[:].bitcast(F32))
                    nc.sync.dma_start(dbg_d["dB3a"][:], B3a[:].bitcast(F32))
                    nc.sync.dma_start(dbg_d["dC3a"][:], C3a[:].bitcast(F32))

            # ================ TAIL (levels 4-6, replicated) ================
            def tail_conv(sm, wts, ins, outs, pg, og, mode, msec, mname):
                N = og * og * og
                noff = wts[0].shape[1]
                offs = OFFSETS if noff == 27 else [(0, 0, 0)]
                for (ot, c0, co_n, padded) in outs:
                    ps = pst.tile([co_n, max(N, 8)], F32, tag="ps")
                    nmm = len(offs) * len(ins)
                    i = 0
                    for o, (dz, dy, dx) in enumerate(offs):
                        for ki, it in enumerate(ins):
                            w = it[:].rearrange("p (z a b) -> p z a b",
                                                z=pg, a=pg)
                            nc.tensor.matmul(
                                ps[:, 0:N].rearrange(
                                    "p (z a b) -> p z a b", z=og, a=og),
                                wts[ki][:, o, c0:c0 + co_n],
                                w[:, 1 + dz:1 + dz + og, 1 + dy:1 + dy + og,
                                  1 + dx:1 + dx + og],
                                start=(i == 0), stop=(i == nmm - 1))
                            i += 1
                    if padded:
                        opg = og + 2
                        dst = ot[:].rearrange("p (z a b) -> p z a b",
                                              z=opg, a=opg)[:, 1:1 + og,
                                                            1:1 + og, 1:1 + og]
                    else:
                        dst = ot[:, 0:N].rearrange("p (z a b) -> p z a b",
                                                   z=og, a=og)
                    src = ps[:, 0:N].rearrange("p (z a b) -> p z a b",
                                               z=og, a=og)
                    if mode == "copy":
                        nc.scalar.copy(dst, src)
                    else:
                        mt = mload(sm, msec, 0, N, co_n, mname)
                        mm = mt[:].rearrange("p (z a b) -> p z a b", z=og, a=og)
                        if mode == "mul":
                            nc.vector.tensor_mul(dst, src, mm)
                        else:
                            nc.vector.tensor_add(dst, src, mm)

            def tail_pool(sm, ss, cs, ps_out, g, has_mask, msec):
                go = g // 2
                for (ct, cn), (pt, _) in zip(cs, ps_out):
                    v = ct[:, 0:g * g * g].rearrange("p (z v) -> p z v",
                                                     v=g * g)
                    t1 = ss.tile([cn, go, g * g], F32, tag=f"tp{g}a")
                    nc.vector.tensor_max(t1[:], v[:, 0::2, :], v[:, 1::2, :])
                    u = t1[:].rearrange("p z (a b) -> p z a b", b=g)
                    t2 = ss.tile([cn, go, go, g], F32, tag=f"tp{g}b")
                    nc.vector.tensor_max(t2[:], u[:, :, 0::2, :],
                                         u[:, :, 1::2, :])
                    gp = go + 2
                    dst = pt[:].rearrange("p (z a b) -> p z a b", z=gp, a=gp)
                    if has_mask:
                        t3 = ss.tile([cn, go, go, go], F32, tag=f"tp{g}c")
                        nc.vector.tensor_max(t3[:], t2[:, :, :, 0::2],
                                             t2[:, :, :, 1::2])
                        mt = mload(sm, msec, 0, go * go * go, cn, f"tp{g}m")
                        nc.vector.tensor_mul(
                            dst[:, 1:1 + go, 1:1 + go, 1:1 + go], t3[:],
                            mt[:].rearrange("p (z a b) -> p z a b",
                                            z=go, a=go))
                    else:
                        nc.vector.tensor_max(
                            dst[:, 1:1 + go, 1:1 + go, 1:1 + go],
                            t2[:, :, :, 0::2], t2[:, :, :, 1::2])

            # ---- L4 ----
            with tc.tile_pool(name="l4w", bufs=1) as wp, \
                 tc.tile_pool(name="l4st", bufs=2) as sp, \
                 tc.tile_pool(name="l4p", bufs=1) as pp, \
                 tc.tile_pool(name="l4s", bufs=2) as ss, \
                 tc.tile_pool(name="l4m", bufs=2) as sm:
                w4c1_t = [wload(wp, sp, "w4c1_0", dt=F32),
                          wload(wp, sp, "w4c1_1", dt=F32)]
                w4c2_t = [wload(wp, sp, "w4c2_0", dt=F32),
                          wload(wp, sp, "w4c2_1", dt=F32)]
                B4a = pp.tile([128, 216], F32); B4b = pp.tile([64, 216], F32)
                C4a = pp.tile([128, 64], F32); C4b = pp.tile([64, 64], F32)
                nc.vector.memset(B4a[:].bitcast(F32), 0.0)
                nc.vector.memset(B4b[:].bitcast(F32), 0.0)
                tail_conv(sm, w4c1_t, [P4a, P4b],
                          [(B4a, 0, 128, True), (B4b, 128, 64, True)], 6, 4,
                          "mul" if mf[4] else "copy", "m4mul", "m4mul")
                tail_conv(sm, w4c2_t, [B4a, B4b],
                          [(C4a, 0, 128, False), (C4b, 128, 64, False)], 6, 4,
                          "add" if mf[4] else "copy", "mn4", "mn4")
                tail_pool(sm, ss, [(C4a, 128), (C4b, 64)],
                          [(P5a, 128), (P5b, 64)], 4, mf[5], "m5p")

                if DBG:
                    nc.sync.dma_start(dbg_d["dP4a"][:], P4a[:])
                    nc.sync.dma_start(dbg_d["dB4a"][:], B4a[:])
                    nc.sync.dma_start(dbg_d["dC4a"][:], C4a[:])

            # ---- L5 ----
            with tc.tile_pool(name="l5w", bufs=1) as wp, \
                 tc.tile_pool(name="l5st", bufs=2) as sp, \
                 tc.tile_pool(name="l5p", bufs=1) as pp, \
                 tc.tile_pool(name="l5s", bufs=2) as ss, \
                 tc.tile_pool(name="l5m", bufs=2) as sm:
                w5c1_t = [wload(wp, sp, "w5c1_0", dt=F32),
                          wload(wp, sp, "w5c1_1", dt=F32)]
                w5c2_t = [wload(wp, sp, "w5c2_0", dt=F32),
                          wload(wp, sp, "w5c2_1", dt=F32)]
                B5a = pp.tile([128, 64], F32); B5b = pp.tile([96, 64], F32)
                C5a = pp.tile([128, 8], F32); C5b = pp.tile([96, 8], F32)
                nc.vector.memset(B5a[:].bitcast(F32), 0.0)
                nc.vector.memset(B5b[:].bitcast(F32), 0.0)
                tail_conv(sm, w5c1_t, [P5a, P5b],
                          [(B5a, 0, 128, True), (B5b, 128, 96, True)], 4, 2,
                          "mul" if mf[5] else "copy", "m5mul", "m5mul")
                tail_conv(sm, w5c2_t, [B5a, B5b],
                          [(C5a, 0, 128, False), (C5b, 128, 96, False)], 4, 2,
                          "add" if mf[5] else "copy", "mn5", "mn5")
                tail_pool(sm, ss, [(C5a, 128), (C5b, 96)],
                          [(P6a, 128), (P6b, 96)], 2, mf[6], "m6p")

                if DBG:
                    nc.sync.dma_start(dbg_d["dP5a"][:], P5a[:])
                    nc.sync.dma_start(dbg_d["dB5a"][:], B5a[:])
                    nc.sync.dma_start(dbg_d["dP6a"][:], P6a[:])

            # ---- L6 (1^3, center tap only) ----
            with tc.tile_pool(name="l6w", bufs=1) as wp, \
                 tc.tile_pool(name="l6st", bufs=2) as sp:
                w6c1_t = [wload(wp, sp, "w6c1_0", dt=F32),
                          wload(wp, sp, "w6c1_1", dt=F32)]
                w6c2_t = [wload(wp, sp, "w6c2_0", dt=F32),
                          wload(wp, sp, "w6c2_1", dt=F32)]
                for (ot, c0) in ((X6a, 0), (X6b, 128)):
                    ps = pst.tile([128, 8], F32, tag="ps")
                    nc.tensor.matmul(ps[:, 0:1], w6c1_t[0][:, 0, c0:c0 + 128],
                                     P6a[:, 13:14], start=True, stop=False)
                    nc.tensor.matmul(ps[:, 0:1], w6c1_t[1][:, 0, c0:c0 + 128],
                                     P6b[:, 13:14], start=False, stop=True)
                    nc.vector.tensor_copy(ot[:], ps[:, 0:1])
                for i, c0 in enumerate((0, 128)):
                    ps = pst.tile([128, 8], F32, tag="ps")
                    nc.tensor.matmul(ps[:, 0:1], w6c2_t[0][:, 0, c0:c0 + 128],
                                     X6a[:], start=True, stop=False)
                    nc.tensor.matmul(ps[:, 0:1], w6c2_t[1][:, 0, c0:c0 + 128],
                                     X6b[:], start=False, stop=True)
                    nc.scalar.copy(outt[:, i:i + 1], ps[:, 0:1])
            if DBG:
                nc.sync.dma_start(dbg_d["dX6a"][:], X6a[:])
            nc.sync.dma_start(out_d[0, 0:128], outt[:, 0])
            nc.sync.dma_start(out_d[0, 128:256], outt[:, 1])

    nc.compile()
    return nc


_CACHE = {}


def kernel(features, coors, W0, W1, W2, W3, W4, W5, W6, W7, W8, W9, W10, W11,
           W12, W13):
    features = np.asarray(features, np.float32)
    coors = np.asarray(coors, np.int32)
    Ws = [np.asarray(w, np.float32) for w in
          (W0, W1, W2, W3, W4, W5, W6, W7, W8, W9, W10, W11, W12, W13)]
    in_maps, meta = build_host_inputs(features, coors, Ws)
    key = tuple(sorted(meta["mask_flags"].items()))
    if key not in _CACHE:
        _CACHE[key] = build_kernel(meta)
    nc = _CACHE[key]
    res = run_bass_kernel_spmd(nc, in_maps, core_ids=list(range(NC)))
    out = res.results[0]["out"].reshape(256)
    return out.reshape(1, 1, 1, 1, 256).astype(np.float32)


if __name__ == "__main__":
    pass
